# revision 40
# baseline (speedup 1.0000x reference)
"""Trainium2 Bass kernel for nn_BCOP (Bjorck-orthonormalized circular conv).

Self-contained: builds weights (power iteration + fitted 8-stage quintic
Newton-Schulz schedule matching the reference's 20 cubic Bjorck iterations)
and the 3x3 circular conv on 8 NeuronCores, data-parallel over batch, with
the per-matrix chains distributed over cores 0-5 and AllGathered.

The 8-stage quintic schedule q_i(x) = a_i x + b_i x^3 + c_i x^5 was fitted
offline so the composition matches the composition of 20 cubic steps
f(x) = 1.5x - 0.5x^3 to max|err| = 1.3e-4 on x in [0, 1.08] (the singular
values after power-iteration scaling live in this interval). Since Newton-
Schulz iterations never rotate singular vectors, matching the scalar map
matches the matrix result.
"""
import base64
import contextlib
import math
import os
import sys

import numpy as np

for _p in ("/opt/trn_rl_repo", "/root/.axon_site/_ro/trn_rl_repo"):
    if _p not in sys.path and os.path.isdir(_p):
        sys.path.insert(0, _p)

import concourse.bacc as bacc
import concourse.bass as bass
import concourse.tile as tile
from concourse import mybir
from concourse.bass_utils import run_bass_kernel_spmd

F32 = mybir.dt.float32
F32R = mybir.dt.float32r
BF16 = mybir.dt.bfloat16
AF = mybir.ActivationFunctionType

NCORES = 8
B, C, H, W = 16, 256, 64, 64
BPC = B // NCORES            # batches per core
NUM_K = 5
USE_CC = os.environ.get("BCOP_USE_CC", "1") == "1"

# Fitted quintic schedule (see module docstring).
SCHED = [
    (+3.537126, -1.143386, +0.178025),
    (+3.486273, -1.233470, +0.114834),
    (+3.524281, -1.330306, +0.137033),
    (+3.425718, -1.406452, +0.147234),
    (+2.932084, -1.646435, +0.250418),
    (+2.298281, -1.697750, +0.420508),
    (+1.621210, -1.184795, +0.380696),
    (+2.043378, -1.577882, +0.524628),
]
NSTAGE = len(SCHED)

NWARM = int(os.environ.get("BCOP_NWARM", "14"))
PI_FILL = int(os.environ.get("BCOP_PI_FILL", "3"))
STAGE_FILL_A = int(os.environ.get("BCOP_SFILL_A", "3"))  # between G and M psum
STAGE_FILL_B = int(os.environ.get("BCOP_SFILL_B", "3"))  # between M psum and WV
GFILL_GROUPS = int(os.environ.get("BCOP_GFILL", "18"))

_U0_B64 = "/wEQPugDtb4BnP0/kWP6PuSbi7/gnwjA+PYNv7jlez+2hrk/+Y+PPtA6Vbxs3x4/+0hcv11wh7/Pq9O/NsCkviaW9j+UZw0/21Rvv94jl79SCki9/QoCPz9/ib+0vce/aajHvnf82z/Gv+o+JRSavyafvbyZUiE/loItv/YVRb/dh02/Wjd+vyiDq7/6VFO+LSBCPsANk77szwY9Ndkgv6E2Nj+02y4/fqQ6P0e+WT+IKJE/l2kHQBSnBT+tJJO/EEb/v5BoAr+4uIQ/izzlPwY70T4Ce7O/u9NDvvRCkT6XeQu9PR0iP+9lN7/ftDa/0QU5v5mfM78iwDK/Lx1Bv3MqTb8FaX+/1Buyv/xXL76+boI+W8ikvWE43z4HZrm/8s2bviaCfj3qVRO/uUh0PwmItT/MUpM+TIzqvANnKT9mokK/tXJLv0YvfL9LGKm/Wh1Pvv5NOD4f1J6+MwxsPQW9/L66eIg//yzRP0Biqz54rvO/iNcLvwLcaT8pZKg/RX3rPaDh6L4hGJk/ZIoIPFi3Gr/0TWI/4H2JP3c71T+K77Q+JJQUwLN9Kb+jgUA/7GpIP4RadT+Akbs/kkyfPqb0Ur2l3wA/KACEvzak5L8nBNS+tFauP1Xobz6lkE++Bj4/Ppgqmb5hyJs9lN0Jv4Tpbj8b0pg/Ci+fO15KHb98sU0/fGCBP8aBCkDeLwk/GuSIvwZLBMA+Yxq/PpFGP1M6OT8ET1Q/4a+JP8OhCUCqvh8/A9s6v4qOPL8CVT+/C8tFv4w5N78YZE+/Mz2Ev6ISJMDdigC/a5BqP7epwj+/XkM++G1WvoOHgz7vUom7PZMGPwEYd7/Ad6O/y/sEu+ENRb/dX1+/y+2Mv3qg+b9D3AG/s7uOP1+LIECyxyg/+j84v/l7ML/9azq/K9Rcv3xRhr/OQ82/OpewvhskDUCvGyg/pBk+v7AVWb88Do2/t9QRwAcJJb8SQjg/Q9osPzXNRD/KDkw/9zNzPyi4wT8SSoE+MQDvvVJh9T7k5KS/LnTpvdu47T7V7Ja/OoycvYveDD8crmS/Yvmhv487D74+xMc+K8jnvwRw376qtrY/48mVPtA+/7tHLB8/rT1Zvz7Wkb8dQgXAcV0Hv+JPjD/od0VAU8YcP4DyYr/sN4a/Y8rIv4B+y76lZd4/oiX3PrHrnr/3Bf29ogO1PjJID8AkLiO/IfY0Px4NMj/Nt0Y/H7RSP5oCbD/tS5M/vlQ6PZQAA79XcZE/bvcAQHj2+j6RSoi/evzOv2tqo741OwVAhOsEP5NEj78XKPG/0KgIvzMlbz8YjZo/Tx17PMARFr8SgU8/owtoP0azpT/0eMI9TjDTvmgMsD8wJyk+yFWEviUTHj73GL2+UHrOP9czpz4QGf+/Hbz7vggLiD/L3s4/lmSoPpyX/79Otvy+6L6HP4zLzT9b/qQ+ZpgGwGQ9Bb9SmY8/J9j0P5xdDT+J4mW/2Uahv/azFL6abcU+IUHpvzYp275a7cA/CFWFPo0eG76EjL4+h6jLv4bOyr4Oudw/1g7XPi/dlL+w5AW+oifVPkeclr9vPhO+2YryPjoWv78dBi++V88vPiz4LL4PIzE+gGkqvlixNj5Z9XG+UFOOPk0B7r2PzLo+YzcEwLiFG78IREQ/HvA1P4qzTD+TG5E/2hHOPwZA0z5HYZm/Ke+sveJpsT4BV+e/ADasvqqExz9uR50/UW+ePVq2/b7PWIw/+kD5Pwl1/D4eU42/Ucb0v8Ut/L7LIoQ/SJnZP/no+T6ZT6W/K4ndvQWz9j6sMKi/K6rFvVro4j71hMO/ubN3vgaesD1I7te+BliyP46aLz7WcoW+jFfxPUpyp76GG/k/oSoOP4Q7b7+l2pa/zYBsvUAdBT8BxpG/FpYIwCHGBL/5lZI/Z0//P83VAD8TXoG/eHDdvxl0+b5hVaE/+nwYPozOur6agtA/wJ2kPj6sAcB21f++/rqJPwHWzT+KSaw+H0v3v1EfFL+ECHk/t/K/PwJmdz717da9I9v0Po+snr8OhAe+3OClPjzABsBWFge/PPqKP7THDEC2eyE/LnQ4v2vpOL8DiDa/MP40v1KvMb8QQkW/Q+9Sv4cTcr/ZZJu/9ILMvMXHJj80AkS/WJBPv2zdab+Rnqa/JCDQvfLC9z5jzp2/cVMCvgN3rz4CjSzAt6QbvxN4YD9jPoc/ExHMP4/MzD4kN9e/PQzlvnoonT96EBQ9YxAjvzDiMz+Dei8/A9k+PxrlXz+RxIU/a8jEP3HJuD6fbNa/x5m3vt69DECJAiM/ZzQzvwMeLb9IFDu/F31Xv9LDjb8nfTbAVRgdv2f9Wz+EYIE/vpnWP5FR5D5255y/hK4PvY70Ij/JADS//IYuvw9JPb8x/1y/EraCv1Ddz7/p6NS+ssuWP6AvFj6Rt+++0frCPwj7Qj7hu1a+ffWAPu40N7zucgE/0R9mv3o9u7+cRm2+TrGUPrsEwr39A6Y+OpfQv0xi1r7LjJQ//CUHPg3b1b4R1ZY/MGkSPj948r6GYb8/IWYrPi3VML7i5CY+DaM8vrg8Zj6aZpm+BJ3dvvG/rz/5Xks+4/dNvqbITj6WTVu+k7lmPhPpJb7JiJ0+7Sonvc57AT9h9YW/hQ/Qv3EquL4P6iRA2vAfP2PRXL9i24e/spbLv8ksu76EVsk/bYfCPlP9078tJKy+uHAJQFAy/T4hMoW//m3Zv2o88L5uy54/Z3kNPvXhvL5oHsc/fSa6PpuUzr97Yau+Phz+P2grED8BsoC/VC2vv+umL76NWnc+E+nKvS5K7z4fT6O/bpvAvaz+0z7dbqm/WsJZvkHXKz4kGXO+zxbMPSVI774Sq6M/KJ+4PYvIzr5qNK8/H+YxPu+Sfr6UPqM9LszZvgsWvT/qp4I+DUHHvWY01T40Xa6/zj5qvmQnVj4xUiy+9Q6BPsrBw72x9c4+JjOyv3o1O75045k+Fp2UvW3DDT/1PWW/EGCiv1sCGr7qlsE+IGnFv8Dmub4Qw9Q/JvS1PokSDMAoHiO/WlE0PzdKLj/Qlj0/IOFePxFhhT/8N+s/CWDgPgi3ub9WIZm+QI+aPbZDC79IxWs/ua+VPwaKYD3RA/6+y52HP7H0yz84bMw+Iyvavz8J674naJY/LZeFPYobEL+tEX4/33+uPxePcj7SV0y+QXxFPt1Ckr71dsY8huspv7GZPz/Vv2E/sWmIPywH0D9HGqo+8ZP4v8dWEL+z8Xw/toysP1zFYz7yziu+DUs0PhJoI74P+0Q+wp5TvsIThD5CAKC9yJsgP+3VU793VYm/7pcFwFxFHL9RC0I/0ewvPwJWXD9KmWQ/JKu4P1z2XT6Hh3O+mm4MPRJZCr/TAoc/Co33Pxd1Jj9wUGG/8Ghwv0CJsL8kHoG+YMNoPHdIAb890Gc/6LO/P6rl3b4ma68/J/lTPs1wXL54S2k+hwFEvoYFiz4Ngd690izPPkdhrb/G41W+yX1EPovmob7N3Ig9tkULv2EvZT8OBKc/08C7PQCa3b7KSrs/Li2RPvovYLyykxs/WclVvz7vkr83F/W/9wARvwx5dj80zrk/KHeVPlqbqryZeCI/s/ctv5p2P79dQ16/byOCv8A34L8zk/S+ECWnP+ow6T2jrOy+yjSXP03Ccj2rwA6/TFl8PxMqrD/YO24+w8VUvp7yIj6rZYW+5C8SPh94yb56q9Y/yjDtPsf6lb/kv5a9UWAMP5QhZr8Jk52/xcEHvr2Xqj62NPO/xcsKvz8/cj8ar5k/uluEPLTnFL9eOVE/mwdtP8YZlz9W/5k9w6kIv/mdbz8xVJg/U1wbPPQXHb+v/14/F+iDPxUP5T8k2tg+WjCrv9HYWL4s/TM+niB4vmN5zT2uy/W+5nCiP7k1ID5fZ7q+huDTP6pDsT6oCynAZF4Zv+6xST96dXk/duXDP+O8jD4xhQK+3MOvPul3MsB0fhy/OSRcP2Imgj+fnts/edTtPsn8k79PFzy9ieMEP+YfkL+/5Pi/gdcPv1OVfT+8uq0/1tNsPtJ7Vr6qPDE+Y8x6vmLR0D3qh/W+axuiP7ezGz6F5b2+xsnMP2TqzT5Vr9a/MFTkvkrynD/tchI9Bicev38RPj8SwEI/xaMxP8bzXz8K4Gs/YPOqP+eOlT6Rdre9uUOrPt3zx7+nUuy+hxmrP6C7lj4tDqy9odayPkKu4r9fmKS+aKXTP5Dh3D6xhqW/3/iKvCpS/z561mu/VEerv5WOl74/B6I9wbS2vrfg2j+T68E+ZGPvv4GGIL/qzZw/C0KXPRrT+76rX4s/85H+PxV9ED+IEWa/k5qcvwCWgr26Wws/+dxjvzL3pL9lseu9j8LvPldhoL+TGwC+CCyoPqjr+r+QbAq/sCBoP/BooT+A8AU+2FeuvrCBDUDPnyc/crQ6vy3TYr/JKoO/1Ozmv5h33b6Y9Lk/Gx6WPjeZpLyWNyk/9TpEv7sUUr/kknC/rmyZv2xkeryl0hg/g45Mv+IVe7+v+6y/oP9ovpt+Sz7W+zm+zeecPv22mb3kgAw/+Jtlv2RLn7+MIwC+n7ezPomVCsDK5yS/ArgwP/7eQz+SGFQ/9TNrP3XNlz9b8nk9/WgOv5opfD/S7Kw/BU5wPoPZSb6Pe0Q+8GCWvs5LWzte7B+/+pdWP6zMjT9BZUZA+pQdP8azWL+9GJC/jkL7vwFaE7/+N3Q/UU62P0yekT6ewce8KxQsP0ulO78BQ1q/DjySv1NbBsBcdgS/lumPP9aJ+T98gBA/7Kt9v9jCrr+oMG++89tRPpKNNr7OtqE+ugNIvZUFBD9ylZG/x/UDwEeTAb+TWoI/3JrcP3Ro8T4i2aW/PZ/LvQYN+D6LHaC/5YAKvonDpD4T7QXAfIoEv1CdkD/+M/0/xE4TPzsfdb/iorq/+ZOdvmzecT1y8BO/E1pzP3kUtz9kzpI+e2SwvIl3Kz9IBDy//i9av4Uhg78VFhjAd9MSv0oljD/dKOE/69nNPhhSCsDNGAm/SfWIP7hvBEDecRo/BXFGv01QOb919FO/EeSIv1tLBcCpNhu/YgRFPw7yNT+Aj0w/heOQP7fHzj+3aNE+/nOav55oxL1sUaM+BW/Uv+9p4L7xsqA/Q4BxPcaRK7/Zxlc/pVhEvxXWXb8CZ42/aSP0v4gGBb+62oI/V2TVP/5TwT7MY92/STbnvlSIlT+UxIw9Y9wRv9NKfz+q+6o/o0RpPladb76k5mk+Ti5gvmZsTD4snSe+i6h9PhBl1r3Jhe4+ibibv42NmrwQWh0/77xev7quhr9t/sy/N26zvqfBJkDAXR4//IhVv8t1jb/rxBzAeiEYvwfqRj8RAng/WVvDP2p3hD7DUBa+cVvJPuLs2L+Vduq+eHaUP6HsUD1edfy+tD2HP91Sxj++LL4+MY7Ev1uSwb5bXMk/hyTNPqhO4b9lhfC+KnOlPxCBpz0GZd2+WpDDPworhD4I/w++jr/HPsII6b9c7t2+yCO/PxeDgj4cjrO975fcPuDsvL9zJXi+ZBrQPc9r+b6WOp8/CdkJPrNopb7uDQlAp50GPyVijb+ZA0/Ar2sev6KZWz8bRpE/4AkCQCyCAT+HqIK/gzHcv4FM8b769aY/yL3TPQTl8r4gJaU/LhHGPWUaz76DE7U/bjZHPn0wk74uQcg8rNAqvyq6PT/XO18/XeuEP5jN6T/Snts+o96/v46kgb7I87M9rsfZvoGowj/uj4k+/FAMvocTpD6yEgfAOMEFvwM1jj+Ww05Ag34fP0+5Vr9Vkoy/gdYdwKjaFb+TulA/PMFpPzRApz/WFt09RujuvlZbkz9j9hw9og4Jv9sfiT87hwVAJOIbP2peQ79otzK/v5BivxtPcb9tCrO/56yHvg9Chz0prSi/pW5eP0Y+aD/4yb4/UdkrPpn1Mr4/ICU+IwFDvmxFVj6CtIO+PPuEO0SNBr/wvHY/Cr+jP74mDjw3rwO/TcZ+P1TgmD9OZ7E9gtOtvr/ZED6WILS+hIf3PwXsAT9Zz5K/YsgRwPx5Fb8iTVg/4MqJP2aJzD9ozsI+5GbQv0aUtL4S/EdATLEXP/2HUb+L0WO/CGGov8R2p72NwdE+oYWsv1qFVr6LMz0+Y4iUvsuL6DyqnSS/nQYzP3T+Nj/w1TA/ggc/P/I5Vj9V9JI/k8n3PxaoEj8A2nK/Ph25v8Atnr73CTc9YnAHvxW9jT+S8RpAJwoYP+jIR79QyHe/LDzCv4/oib5qhvk9xEayvlcYREABFBw/E+lfvw8Fib9yic+/4cqlvqiCBUAnPQY/6luKv68XGsDj0ie/1cZBPw3jSD//M3U/8pa4P6jAjz5WW6i8rXoqP2mpP79muGC/mgOGvzqGx79z2r6+HHzJP3+sxz6AZua/5gHWvuCVrj93S2o+I01XvnBzMj64xXW+b9rpPbkr5r50K50/eKbyPDf8I7/hXzA/lllAP5A/SD+QpXQ/oVS6P7DKmz6JyYO9OfEPPy3ufL/bZqy/pJdfvsc6cD4nh1O+m/E5PiVNoL4jc1o9iEsBv+0OhD+Ij+Q/CzDUPt+Lrb8ZzGi+lmhcPvwiJb4JNok+IQcLvlNvpT7DhATAot8Cv1L9kj8g9ghAU30HP/p0i79aLxPAS00ov8TuQj+djEw/ohOAP5+gsT9aljI+GHx4vsK62z3wxO++b+aTP/EP/D1u/N2+oWyjP7uqXDsCqAa/AoB3P4ddpD+6dSY8FXICv/jCZD/3gLg/GTZZPqL9er6koLY8uFsSv34rjT9xqec//xitPpnXxb9ymOi+h0avP6m7dD4ApBa9DpoIP7jxiL8ZOgbAx3Ycvz1HQz8ubTE/d5dgPx1Kbj8="


def _u0():
    return np.frombuffer(base64.b64decode(_U0_B64), dtype="<f4").reshape(5, 256).copy()


def _mm256p(nc, ps512, terms):
    """[256,256] matmul sum into ONE [128,512] psum bank as a single
    accumulation group: result rows m*128..+128 at cols m*256..+256."""
    n = 0
    total = len(terms) * 4
    for m in range(2):
        for lhsT_tiles, rhs_tiles in terms:
            for kt in range(2):
                nc.tensor.matmul(
                    ps512[:, m * 256:(m + 1) * 256],
                    lhsT_tiles[kt][:, m * 128:(m + 1) * 128],
                    rhs_tiles[kt][:],
                    start=n == 0,
                    stop=n == total - 1,
                )
                n += 1


def _mm256(nc, psums, terms):
    """[256,256] matmul sum over terms: psums[m] += sum_p lhsT_p.T @ rhs_p."""
    for m in range(2):
        for pi, (lhsT_tiles, rhs_tiles) in enumerate(terms):
            for kt in range(2):
                nc.tensor.matmul(
                    psums[m][:],
                    lhsT_tiles[kt][:, m * 128:(m + 1) * 128],
                    rhs_tiles[kt][:],
                    start=pi == 0 and kt == 0,
                    stop=pi == len(terms) - 1 and kt == 1,
                )


def _mv256(nc, ps2, lhsT_tiles, z_tiles):
    """matvec into one [128,2] psum tile: half m lands in column m."""
    n = 0
    for m in range(2):
        for kt in range(2):
            nc.tensor.matmul(
                ps2[:, m:m + 1],
                lhsT_tiles[kt][:, m * 128:(m + 1) * 128],
                z_tiles[kt],
                start=n == 0,
                stop=n == 3,
            )
            n += 1


def build_nc(use_cc=USE_CC):
    nc = bacc.Bacc("TRN2", target_bir_lowering=False, debug=False,
                   num_devices=NCORES)

    n_mat_local = 1 if use_cc else NUM_K

    x_in = nc.dram_tensor("x", [BPC, C, H + 2, W + 2], BF16,
                           kind="ExternalInput")
    pm_in = nc.dram_tensor("pm", [n_mat_local, C, C], F32, kind="ExternalInput")
    pmT_in = nc.dram_tensor("pmT", [n_mat_local, C, C], F32, kind="ExternalInput")
    u0_in = nc.dram_tensor("u0", [n_mat_local, C, 1], F32, kind="ExternalInput")
    eye_in = nc.dram_tensor("eye1", [C, C], F32, kind="ExternalInput")
    eyec_in = nc.dram_tensor("eyecat", [128, 512], F32, kind="ExternalInput")
    bias_in = nc.dram_tensor("biasc", [C, 1], F32, kind="ExternalInput")
    out_dram = nc.dram_tensor("out", [BPC, C, H, W], F32, kind="ExternalOutput")

    with tile.TileContext(nc) as tc, contextlib.ExitStack() as top:
        const = top.enter_context(tc.tile_pool(name="const", bufs=1))
        xpool = top.enter_context(tc.tile_pool(name="xpool", bufs=1))
        tpool = top.enter_context(tc.tile_pool(name="tpool", bufs=1))
        v5pool = top.enter_context(tc.tile_pool(name="v5pool", bufs=1))
        vfin = top.enter_context(tc.tile_pool(name="vfin", bufs=1))

        eye = [const.tile([128, 256], F32, name=f"eye_{t}", tag=f"eye{t}")
               for t in range(2)]
        bias_c = [const.tile([128, 1], F32, name=f"bias_{t}", tag=f"bias{t}")
                  for t in range(2)]
        ones_f = const.tile([1, 128], F32, name="ones_f", tag="onesf")
        eyec = const.tile([128, 512], F32, name="eyec", tag="eyec")
        nc.sync.dma_start(eyec[:], eyec_in[:])
        for t in range(2):
            nc.sync.dma_start(eye[t][:], eye_in[t * 128:(t + 1) * 128, :])
            nc.sync.dma_start(bias_c[t][:], bias_in[t * 128:(t + 1) * 128, :])
        nc.any.memset(ones_f[:], 1.0)

        # ---- chain inputs staged first so their DMAs beat the big x DMAs ----
        chain_in = []
        cinp = top.enter_context(tc.tile_pool(name="cinp", bufs=1))
        for mi in range(n_mat_local):
            A = [cinp.tile([128, 256], F32, name=f"A_{mi}_{t}", tag=f"A{mi}{t}")
                 for t in range(2)]
            AT = [cinp.tile([128, 256], F32, name=f"AT_{mi}_{t}", tag=f"AT{mi}{t}")
                  for t in range(2)]
            z0 = [cinp.tile([128, 1], F32, name=f"z0_{mi}_{t}", tag=f"z0{mi}{t}")
                  for t in range(2)]
            for t in range(2):
                nc.sync.dma_start(A[t][:], pm_in[mi, t * 128:(t + 1) * 128, :])
                nc.sync.dma_start(AT[t][:], pmT_in[mi, t * 128:(t + 1) * 128, :])
                nc.sync.dma_start(z0[t][:], u0_in[mi, t * 128:(t + 1) * 128, :])
            chain_in.append((A, AT, z0))

        # per-stage scaled-identity lhsT for the b*G psum term:
        #   bI[i] = (b_i/sqrt(c_i)) * I128
        aeye0 = const.tile([128, 512], F32, name="aeye0", tag="aeye0")
        nc.scalar.mul(aeye0[:], eyec[:], float(SCHED[0][0]))
        bI = []
        for i, (a_i, b_i, c_i) in enumerate(SCHED):
            bi = const.tile([128, 128], F32R, name=f"bI_{i}", tag=f"bI{i}")
            nc.vector.tensor_scalar_mul(bi[:], eye[0][:, 0:128],
                                        float(b_i / math.sqrt(c_i)))
            bI.append(bi)

        # ---- early barrier: tiny AllGather aligns core start times so the
        # real gather's multi-phase mesh sees ~zero skew ---------------------
        barp = top.enter_context(tc.tile_pool(name="barp", bufs=1,
                                              space="DRAM"))
        if use_cc:
            # fire-and-forget: forces CC-ring/peer init to overlap the chain
            # so the real gather is the (fast) second collective
            bsrc = const.tile([128, 1], F32, name="bsrc", tag="bsrc")
            nc.vector.memset(bsrc[:], 1.0)
            bgin = barp.tile([128, 1], F32, name="bgin", tag="bgin")
            bgout = barp.tile([NCORES, 128, 1], F32, name="bgout", tag="bgout")
            nc.sync.dma_start(bgin[:], bsrc[:])
            nc.gpsimd.collective_compute(
                "AllGather", mybir.AluOpType.bypass,
                replica_groups=[list(range(NCORES))],
                ins=[bgin.opt()], outs=[bgout.opt()],
            )

        # ---- PE warmup burst (gated on the barrier) to lift the clock ------
        wsrc = const.tile([128, 512], F32, name="wsrc", tag="wsrc")
        nc.vector.memset(wsrc[:], 1.0)
        dummy2 = const.tile([128, 512], F32R, name="dummy2", tag="dummy2")
        nc.vector.tensor_copy(dummy2[:], wsrc[:])
        dummy_r = const.tile([128, 256], F32R, name="dummy_r", tag="dummyr")
        nc.scalar.copy(dummy_r[:], wsrc[:, 0:256])
        with tc.tile_pool(name="warmps", bufs=1, space="PSUM") as wps:
            wp_ = wps.tile([128, 512], F32, name="warm", tag="warm")
            for wi in range(NWARM):
                nc.tensor.matmul(wp_[:], dummy_r[:, 0:128], dummy2[:],
                                 start=wi == 0, stop=wi == NWARM - 1)

        # ---- x load: direct bf16 DMA into padded tile + tiny edge copies ----
        Xp = [[xpool.tile([128, 66, 66], BF16, name=f"xp_{b}_{g}",
                          tag=f"xp{b}{g}") for g in range(2)]
              for b in range(BPC)]
        for b in range(BPC):
            for g in range(2):
                nc.sync.dma_start(Xp[b][g][:],
                                  x_in[b, g * 128:(g + 1) * 128, :, :])

        # ---- per-matrix chain: power iteration + fitted quintic schedule ----
        def half(tile512):
            return [tile512[:, t * 256:(t + 1) * 256] for t in range(2)]

        def fill(tag_idx, n, free=256):
            for fi in range(n):
                fl = cps.tile([1, free], F32, name="fl", tag="fl0", bufs=1)
                nc.tensor.matmul(fl[:], dummy_r[0:128, 0:1],
                                 dummy2[:, 0:free], start=True, stop=True)

        def mmh(ps_half, m, lhsT_tiles, rhs_tiles, extra_bI=None):
            """Half-row product: ps_half = (lhsT^T rhs) rows m*128..+128,
            optionally + bI^T rhs_m as a 3rd matmul in the same group."""
            total = 2 + (1 if extra_bI is not None else 0)
            for kt in range(2):
                nc.tensor.matmul(ps_half[:],
                                 lhsT_tiles[kt][:, m * 128:(m + 1) * 128],
                                 rhs_tiles[kt][:],
                                 start=kt == 0, stop=kt == 1 and total == 2)
            if extra_bI is not None:
                nc.tensor.matmul(ps_half[:], extra_bI[:], rhs_tiles[m][:],
                                 start=False, stop=True)

        v_final = []
        with tc.tile_pool(name="chain", bufs=2) as cp, \
             tc.tile_pool(name="chps", bufs=2, space="PSUM") as cps:
            for mi in range(n_mat_local):
                pt = mi % 2
                A, AT, z0 = chain_in[mi]

                # fp32r copies of A / A^T (used by PI and as stage-0 W/V)
                Ar_sb = cp.tile([128, 512], F32R, name="Ar", tag=f"Ar{pt}",
                                bufs=1)
                ATr_sb = cp.tile([128, 512], F32R, name="ATr", tag=f"ATr{pt}",
                                 bufs=1)
                for t in range(2):
                    nc.vector.tensor_copy(Ar_sb[:, t * 256:(t + 1) * 256],
                                          A[t][:])
                    nc.scalar.copy(ATr_sb[:, t * 256:(t + 1) * 256], AT[t][:])
                Ar, ATr = half(Ar_sb), half(ATr_sb)

                # Deferred-normalization power iteration (replicates the
                # reference's 10 normalized PI rounds up to fp noise):
                # G2 = A A^T, G4 = G2^2, G8 = G4^2; z9 = G8 G2 z0;
                # y = A^T z9; zf = A y; s = sqrt(|zf|^2/|y|^2).
                # Stage-0's unscaled G-hat = A^T A is interleaved here; the
                # 1/s^2 and 1/s factors fold into stage-0's copies.
                g2h = [cps.tile([128, 256], F32, name=f"g2_{m}",
                                tag=("hA", "hB")[m]) for m in range(2)]
                for m in range(2):
                    mmh(g2h[m], m, AT, AT)
                G2_sb = cp.tile([128, 512], F32, name="G2_sb", tag=f"G2{pt}",
                                bufs=1)
                nc.scalar.copy(G2_sb[:, 0:256], g2h[0][:])
                nc.vector.tensor_copy(G2_sb[:, 256:512], g2h[1][:])
                G2 = half(G2_sb)
                # stage-0 G-hat (independent of the scale chain)
                gph = [cps.tile([128, 256], F32, name=f"gp_{m}",
                                tag=("hC", "hD")[m], bufs=1) for m in range(2)]
                for m in range(2):
                    mmh(gph[m], m, Ar, Ar)
                g4h = [cps.tile([128, 256], F32, name=f"g4_{m}",
                                tag=("hA", "hB")[m]) for m in range(2)]
                for m in range(2):
                    mmh(g4h[m], m, G2, G2)
                Gu_sb = cp.tile([128, 512], F32R, name="Gu_sb",
                                tag=f"Gu{pt}", bufs=1)
                nc.scalar.copy(Gu_sb[:, 0:256], gph[0][:])
                nc.vector.tensor_copy(Gu_sb[:, 256:512], gph[1][:])
                Gu = half(Gu_sb)
                G4_sb = cp.tile([128, 512], F32, name="G4_sb", tag=f"G4{pt}",
                                bufs=1)
                nc.scalar.copy(G4_sb[:, 0:256], g4h[0][:])
                nc.vector.tensor_copy(G4_sb[:, 256:512], g4h[1][:])
                G4 = half(G4_sb)
                # z1 = G2 z0 while G8 is on the PE
                zp = cps.tile([128, 2], F32, name="zp", tag="pv0", bufs=1)
                _mv256(nc, zp, G2, z0)
                zs = cp.tile([128, 2], F32, name="zs", tag=f"z{pt}")
                nc.vector.tensor_copy(zs[:], zp[:])
                z = [zs[:, 0:1], zs[:, 1:2]]
                g8h = [cps.tile([128, 256], F32, name=f"g8_{m}",
                                tag=("hA", "hB")[m]) for m in range(2)]
                for m in range(2):
                    mmh(g8h[m], m, G4, G4)
                G8_sb = cp.tile([128, 512], F32, name="G8_sb", tag=f"G8{pt}",
                                bufs=1)
                nc.scalar.copy(G8_sb[:, 0:256], g8h[0][:])
                nc.vector.tensor_copy(G8_sb[:, 256:512], g8h[1][:])
                G8 = half(G8_sb)
                g2u = [cps.tile([128, 256], F32, name=f"g2u_{m}",
                                tag=("hA", "hB")[m]) for m in range(2)]
                for m in range(2):
                    mmh(g2u[m], m, Gu, Gu)
                fill(0, PI_FILL)
                # z9 = G8 z1 ; y = A^T z9 ; zf = A y
                zp = cps.tile([128, 2], F32, name="zp9", tag="pv0", bufs=1)
                _mv256(nc, zp, G8, z)
                zs = cp.tile([128, 2], F32, name="zs9", tag=f"z9{pt}")
                nc.vector.tensor_copy(zs[:], zp[:])
                z = [zs[:, 0:1], zs[:, 1:2]]
                fill(1, 2)
                yp = cps.tile([128, 2], F32, name="yp", tag="pv0", bufs=1)
                _mv256(nc, yp, A, z)
                ys = cp.tile([128, 2], F32, name="ys", tag=f"y{pt}")
                nc.vector.tensor_copy(ys[:], yp[:])
                y = [ys[:, 0:1], ys[:, 1:2]]
                fill(0, 2)
                zp = cps.tile([128, 2], F32, name="zfp", tag="pv0", bufs=1)
                _mv256(nc, zp, AT, y)
                zs = cp.tile([128, 2], F32, name="zfs", tag=f"zf{pt}")
                nc.vector.tensor_copy(zs[:], zp[:])
                z = [zs[:, 0:1], zs[:, 1:2]]
                fill(1, 2)

                # ny = y.y ; nz = z.z ; inv_s = sqrt(ny/nz) bcast;
                # inv_s2 = ny/nz bcast
                nyp = cps.tile([128, 1], F32, name="nyp", tag="pv0", bufs=1)
                for t in range(2):
                    nc.tensor.matmul(nyp[0:1, 0:1], y[t], y[t],
                                     start=t == 0, stop=t == 1)
                nzp = cps.tile([128, 1], F32, name="nzp", tag="pv0", bufs=1)
                for t in range(2):
                    nc.tensor.matmul(nzp[0:1, 0:1], z[t], z[t],
                                     start=t == 0, stop=t == 1)
                sc = cp.tile([1, 6], F32, name="sc", tag=f"sc{pt}")
                nc.vector.tensor_copy(sc[:, 0:1], nyp[0:1, :])
                nc.vector.reciprocal(sc[:, 1:2], nzp[0:1, :])
                nc.vector.tensor_scalar_mul(sc[:, 2:3], sc[:, 0:1],
                                            sc[:, 1:2])
                nc.scalar.sqrt(sc[:, 3:4], sc[:, 2:3])
                nc.vector.tensor_scalar_mul(sc[:, 4:5], sc[:, 2:3],
                                            float(SCHED[0][1]))
                nc.vector.tensor_scalar(sc[:, 5:6], sc[:, 2:3], sc[:, 2:3],
                                        float(SCHED[0][2]),
                                        op0=mybir.AluOpType.mult,
                                        op1=mybir.AluOpType.mult)
                bcp = cps.tile([128, 3], F32, name="bcp", tag="pv0", bufs=1)
                nc.tensor.matmul(bcp[:], ones_f[:], sc[:, 3:6],
                                 start=True, stop=True)
                inv_b = cp.tile([128, 3], F32, name="inv_b", tag=f"invb{pt}")
                nc.scalar.copy(inv_b[:], bcp[:])
                inv_s = inv_b[:, 0:1]    # 1/s broadcast
                b0_bc = inv_b[:, 1:2]    # b_0/s^2 broadcast
                c0_bc = inv_b[:, 2:3]    # c_0/s^4 broadcast

                w, v = Ar, ATr
                for it in range(NSTAGE):
                    a_i, b_i, c_i = SCHED[it]
                    sq_i = float(math.sqrt(c_i))
                    last = it == NSTAGE - 1
                    if it == 0:
                        gph_it = gph   # precomputed A^T A (unscaled)
                    else:
                        gph_it = [cps.tile([128, 256], F32, name=f"gp_{m}",
                                           tag=("hC", "hD")[m], bufs=1)
                                  for m in range(2)]
                        for m in range(2):
                            mmh(gph_it[m], m, w, w)
                    M_sb = cp.tile([128, 512], F32R, name="M_sb", tag=f"M{pt}")
                    if it == 0:
                        # M = a0*I + (b0/s^2)*Ghat + (c0/s^4)*Ghat^2, built
                        # straight from the unscaled psums with runtime bcasts
                        u1 = cp.tile([128, 512], F32, name="u1",
                                     tag=f"u1{pt}", bufs=1)
                        for m in range(2):
                            h = slice(m * 256, (m + 1) * 256)
                            nc.vector.scalar_tensor_tensor(
                                u1[:, h], gph_it[m][:], b0_bc, aeye0[:, h],
                                op0=mybir.AluOpType.mult,
                                op1=mybir.AluOpType.add)
                            nc.vector.scalar_tensor_tensor(
                                M_sb[:, h], g2u[m][:], c0_bc, u1[:, h],
                                op0=mybir.AluOpType.mult,
                                op1=mybir.AluOpType.add)
                    else:
                        Gs_sb = cp.tile([128, 512], F32R, name="Gs_sb",
                                        tag=f"G{pt}")
                        nc.scalar.mul(Gs_sb[:, 0:256], gph_it[0][:], sq_i)
                        nc.vector.tensor_scalar_mul(Gs_sb[:, 256:512],
                                                    gph_it[1][:], sq_i)
                        Gs = half(Gs_sb)
                        fill(it, STAGE_FILL_A)
                        # mp = c*G^2 + b*G per half (separate groups)
                        mph = [cps.tile([128, 256], F32, name=f"mp_{m}",
                                        tag=("hA", "hB")[m]) for m in range(2)]
                        for m in range(2):
                            mmh(mph[m], m, Gs, Gs, extra_bI=bI[it])
                        # M = a_i*eyec + mp per half
                        nc.vector.scalar_tensor_tensor(
                            M_sb[:, 0:256], eyec[:, 0:256], float(a_i),
                            mph[0][:],
                            op0=mybir.AluOpType.mult, op1=mybir.AluOpType.add)
                        nc.vector.scalar_tensor_tensor(
                            M_sb[:, 256:512], eyec[:, 256:512], float(a_i),
                            mph[1][:],
                            op0=mybir.AluOpType.mult, op1=mybir.AluOpType.add)
                    M = half(M_sb)
                    fill(it + 1, STAGE_FILL_B)
                    if last and use_cc:
                        # Only V'[0:128,:] feeds the gather.
                        vph = cps.tile([128, 256], F32, name="vp2", tag="hC",
                                       bufs=1)
                        for kt in range(2):
                            nc.tensor.matmul(vph[:],
                                             M[kt][:, 0:128], v[kt],
                                             start=kt == 0, stop=kt == 1)
                        v_sb = vfin.tile([128, 512], F32R, name=f"vf_{mi}",
                                         tag=f"vf{mi}")
                        nc.vector.tensor_copy(v_sb[:, 0:256], vph[:])
                        v = half(v_sb)
                        v_final.append(v)
                        break
                    # W' = W M (lhsT = V), V' = M V (lhsT = M), per half
                    wph = [cps.tile([128, 256], F32, name=f"wp_{m}",
                                    tag=("hA", "hB")[m]) for m in range(2)]
                    vph = [cps.tile([128, 256], F32, name=f"vp_{m}",
                                    tag=("hC", "hD")[m], bufs=1) for m in range(2)]
                    for m in range(2):
                        mmh(wph[m], m, v, M)
                    for m in range(2):
                        mmh(vph[m], m, M, v)
                    fill(it, 2)
                    pool = vfin if last else cp
                    w_sb = cp.tile([128, 512], F32R, name="w_sb", tag=f"w{pt}")
                    v_sb = pool.tile([128, 512], F32R,
                                     name=f"vf_{mi}" if last else "v_sb",
                                     tag=f"vf{mi}" if last else f"v{pt}")
                    if it == 0:
                        nc.scalar.mul(w_sb[:, 0:256], wph[0][:], inv_s)
                        nc.vector.tensor_scalar_mul(w_sb[:, 256:512],
                                                    wph[1][:], inv_s)
                        nc.scalar.mul(v_sb[:, 0:256], vph[0][:], inv_s)
                        nc.vector.tensor_scalar_mul(v_sb[:, 256:512],
                                                    vph[1][:], inv_s)
                    else:
                        nc.scalar.copy(w_sb[:, 0:256], wph[0][:])
                        nc.vector.tensor_copy(w_sb[:, 256:512], wph[1][:])
                        nc.scalar.copy(v_sb[:, 0:256], vph[0][:])
                        nc.vector.tensor_copy(v_sb[:, 256:512], vph[1][:])
                    w, v = half(w_sb), half(v_sb)
                else:
                    v_final.append(v)

        # ---- gather the needed V halves across cores ------------------------
        # Rank r carries matrix [1,2,3,4, 0, 0*Q, 1, 2][r], where Q swaps
        # column halves - ortho(A Q) = ortho(A) Q, so rank 5's V[:128]
        # equals V_0[128:256].
        V0 = [None, None]
        Vh = [None] * 4
        if use_cc:
            with tc.tile_pool(name="ccdram", bufs=1, space="DRAM") as dp, \
                 tc.tile_pool(name="vstg", bufs=4) as vstg, \
                 tc.tile_pool(name="fillps", bufs=2, space="PSUM") as fps:
                gin = dp.tile([128, C], BF16, name="gin", tag="gin")
                gout = dp.tile([NCORES, 128, C], BF16, name="gout", tag="gout")
                vbf = vstg.tile([128, C], BF16, name="vbf", tag="vbf")
                nc.vector.tensor_copy(vbf[:], v_final[0][0])
                nc.sync.dma_start(gin[:, :], vbf[:])
                gate = vstg.tile([128, 1], F32R, name="gate", tag="gate")
                nc.vector.tensor_copy(gate[:], v_final[0][0][:, 0:1])
                nc.gpsimd.collective_compute(
                    "AllGather", mybir.AluOpType.bypass,
                    replica_groups=[list(range(NCORES))],
                    ins=[gin.opt()], outs=[gout.opt()],
                )
                for gi in range(GFILL_GROUPS):
                    fl = fps.tile([1, 512], F32, name=f"gfill_{gi}",
                                  tag=f"gfill{gi % 2}")
                    for wi in range(6):
                        nc.tensor.matmul(fl[:], gate[:, 0:1], dummy2[:],
                                         start=wi == 0, stop=wi == 5)
                for slot, dest in [(0, ("vh", 0)), (1, ("vh", 1)),
                                   (2, ("vh", 2)), (3, ("vh", 3)),
                                   (4, ("v0", 0)), (5, ("v0", 1))]:
                    kind, idx = dest
                    vs = v5pool.tile([128, 256], BF16, name=f"{kind}_{idx}",
                                     tag=f"{kind}{idx}")
                    nc.sync.dma_start(vs[:], gout[slot, :, :])
                    if kind == "vh":
                        Vh[idx] = vs
                    else:
                        V0[idx] = vs
        else:
            for t in range(2):
                v0 = v5pool.tile([128, 256], BF16, name=f"v50_{t}", tag=f"v50{t}")
                nc.vector.tensor_copy(v0[:], v_final[0][t])
                V0[t] = v0
            for b in range(4):
                vh = v5pool.tile([128, 256], BF16, name=f"vh_{b}", tag=f"vh{b}")
                nc.vector.tensor_copy(vh[:], v_final[1 + b][0])
                Vh[b] = vh

        # ---- tail: PQ, block_orth pair products, matrix_conv, T -------------
        Ttap = [[[tpool.tile([128, 256], BF16, name=f"T_{k}_{l}_{t}",
                             tag=f"T{k}{l}{t}")
                  for t in range(2)] for l in range(3)] for k in range(3)]
        with tc.tile_pool(name="tail", bufs=1) as tl, \
             tc.tile_pool(name="tailps", bufs=1, space="PSUM") as tps:
            if use_cc:
                for wi in range(2):
                    twf = tps.tile([1, 256], F32, name=f"twf_{wi}",
                                   tag="twf0")
                    nc.tensor.matmul(twf[:], Vh[0][:, 0:1], Vh[0][:],
                                     start=True, stop=True)
            PQ = []
            for b in range(4):
                pq = [tl.tile([128, 256], F32R, name=f"pq_{b}_{t}",
                              tag=f"pq{b}{t}") for t in range(2)]
                ps = tps.tile([128, 512], F32, name="pqps", tag="pqps")
                for m in range(2):
                    nc.tensor.matmul(ps[:, m * 256:(m + 1) * 256],
                                     Vh[b][:, m * 128:(m + 1) * 128],
                                     Vh[b][:],
                                     start=True, stop=True)
                for m in range(2):
                    nc.scalar.copy(pq[m][:], ps[:, m * 256:(m + 1) * 256])
                del ps
                PQ.append(pq)

            def pair_products(pa, pb, name):
                """e[c][r]: [0][0]=pa@pb, [0][1]=pa-C, [1][0]=pb-C,
                [1][1]=I-pa-pb+C (symmetric projection algebra)."""
                ps = [tps.tile([128, 256], F32, name=f"ccps_{t}", tag=f"ccps{t}")
                      for t in range(2)]
                _mm256(nc, ps, [(pa, pb)])
                e = [[[tl.tile([128, 256], F32R, name=f"{name}_e{i}{j}_{t}",
                               tag=f"{name}e{i}{j}{t}")
                       for t in range(2)] for j in range(2)] for i in range(2)]
                q = [tl.tile([128, 256], F32, name=f"{name}_q_{t}",
                             tag=f"{name}q{t}") for t in range(2)]
                for t in range(2):
                    nc.scalar.copy(e[0][0][t][:], ps[t][:])
                    nc.vector.tensor_sub(e[0][1][t][:], pa[t][:],
                                         e[0][0][t][:].bitcast(F32))
                    nc.vector.tensor_sub(e[1][0][t][:], pb[t][:],
                                         e[0][0][t][:].bitcast(F32))
                    nc.vector.tensor_sub(q[t][:], eye[t][:],
                                         pa[t][:].bitcast(F32))
                    nc.vector.tensor_sub(e[1][1][t][:], q[t][:],
                                         e[1][0][t][:].bitcast(F32))
                return e

            # m1T[c1][r1] = a2[c1] a1[r1]; m2[r2][c2] = a3[r2] a4[c2]
            m1T = pair_products(PQ[1], PQ[0], "m1T")
            m2 = pair_products(PQ[2], PQ[3], "m2")

            with tc.tile_pool(name="p3pool", bufs=3) as p3p:
                for i in range(3):
                    for j in range(3):
                        terms = [(i1, j1) for i1 in range(min(2, i + 1))
                                 for j1 in range(min(2, j + 1))
                                 if i - i1 < 2 and j - j1 < 2]
                        ps = [tps.tile([128, 256], F32, name=f"p3ps_{t}",
                                       tag=f"p3ps{t}") for t in range(2)]
                        _mm256(nc, ps, [(m1T[j1][i1], m2[i - i1][j - j1])
                                        for (i1, j1) in terms])
                        cell = [p3p.tile([128, 256], BF16, name=f"cell_{t}",
                                         tag=f"cell{t}") for t in range(2)]
                        for t in range(2):
                            nc.scalar.copy(cell[t][:], ps[t][:])
                        tp = [tps.tile([128, 256], F32, name=f"tps_t{t}",
                                       tag=f"tpsT{t}") for t in range(2)]
                        _mm256(nc, tp, [(V0, cell)])
                        for t in range(2):
                            nc.scalar.copy(Ttap[i][j][t][:], tp[t][:])

        # ---- conv: out[o, pix] += T[kw][kh][i, o] * Xp[i, pix+tap] ----------
        with tc.tile_pool(name="ops", bufs=8, space="PSUM") as ops, \
             tc.tile_pool(name="ostg", bufs=8) as ostg:
            for b in range(BPC):
                for ot in range(2):
                    for q in range(4):
                        ptiles = [ops.tile([128, 512], F32, name=f"cps_{k}",
                                           tag="convps") for k in range(2)]
                        first, last = (0, 0), (8, 1)
                        for tap in range(9):
                            kh, kw = tap // 3, tap % 3
                            for kt in range(2):
                                lhs = Ttap[kw][kh][kt][:, ot * 128:(ot + 1) * 128]
                                for k in range(2):
                                    h0 = q * 16 + k * 8
                                    rhs = Xp[b][kt][:, h0 + kh:h0 + kh + 8,
                                                    kw:kw + 64]
                                    nc.tensor.matmul(
                                        ptiles[k][:], lhs, rhs,
                                        start=(tap, kt) == first,
                                        stop=(tap, kt) == last)
                        for k in range(2):
                            h0 = q * 16 + k * 8
                            so = ostg.tile([128, 512], F32, name="so",
                                           tag="ostg")
                            if k == 0:
                                nc.scalar.activation(
                                    so[:], ptiles[k][:], AF.Identity,
                                    bias=bias_c[ot][:], scale=1.0)
                            else:
                                nc.vector.tensor_scalar_add(
                                    so[:], ptiles[k][:], bias_c[ot][:])
                            nc.sync.dma_start(
                                out_dram[b, ot * 128:(ot + 1) * 128,
                                         h0:h0 + 8, :].rearrange(
                                             "c h w -> c (h w)"),
                                so[:])

    nc.compile()
    return nc


_CACHE = {}


def _get_nc():
    key = (USE_CC,)
    if key not in _CACHE:
        _CACHE[key] = build_nc(USE_CC)
    return _CACHE[key]


def make_in_maps(x, param_matrices, bias, use_cc=None):
    if use_cc is None:
        use_cc = USE_CC
    import ml_dtypes
    x = np.ascontiguousarray(x, dtype=np.float32).astype(ml_dtypes.bfloat16)
    x = np.pad(x, ((0, 0), (0, 0), (1, 1), (1, 1)), mode="wrap")
    pm = np.ascontiguousarray(param_matrices, dtype=np.float32)
    bias = np.ascontiguousarray(bias, dtype=np.float32)
    u0 = _u0()
    eye1 = np.eye(C, dtype=np.float32)
    eyecat = np.zeros((128, 512), np.float32)
    for t in range(2):
        eyecat[:, t * 256 + t * 128:t * 256 + t * 128 + 128] = np.eye(128)
    bias_c = bias.reshape(C, 1)
    pmT = np.ascontiguousarray(pm.transpose(0, 2, 1))
    in_maps = []
    perm = np.concatenate([np.arange(128, 256), np.arange(128)])
    pm0q = np.ascontiguousarray(pm[0][:, perm])
    pm0qT = np.ascontiguousarray(pm0q.T)
    sel_map = [1, 2, 3, 4, 0, -1, 1, 2]   # -1 = matrix 0 * Q
    for c in range(NCORES):
        if use_cc:
            sel = sel_map[c]
            if sel == -1:
                pm_l = pm0q[None]
                pmT_l = pm0qT[None]
                u0_l = u0[0:1, :, None]
            else:
                pm_l = pm[sel:sel + 1]
                pmT_l = pmT[sel:sel + 1]
                u0_l = u0[sel:sel + 1, :, None]
        else:
            pm_l, pmT_l, u0_l = pm, pmT, u0[:, :, None]
        in_maps.append({
            "x": x[c * BPC:(c + 1) * BPC],
            "pm": np.ascontiguousarray(pm_l),
            "pmT": np.ascontiguousarray(pmT_l),
            "u0": np.ascontiguousarray(u0_l),
            "eye1": eye1,
            "eyecat": eyecat,
            "biasc": bias_c,
        })
    return in_maps


def kernel(x, param_matrices, bias, _trace=False):
    nc = _get_nc()
    in_maps = make_in_maps(x, param_matrices, bias)
    res = run_bass_kernel_spmd(nc, in_maps, list(range(NCORES)), trace=_trace)
    out = np.concatenate([res.results[c]["out"] for c in range(NCORES)], axis=0)
    if _trace:
        kernel._last_result = res
    return out


# revision 41
# speedup vs baseline: 1.2758x; 1.2758x over previous
"""Trainium2 Bass kernel for nn_BCOP (Bjorck-orthonormalized circular conv).

Self-contained: builds weights (power iteration + fitted 8-stage quintic
Newton-Schulz schedule matching the reference's 20 cubic Bjorck iterations)
and the 3x3 circular conv on 8 NeuronCores, data-parallel over batch, with
the per-matrix chains distributed over cores 0-5 and AllGathered.

The 8-stage quintic schedule q_i(x) = a_i x + b_i x^3 + c_i x^5 was fitted
offline so the composition matches the composition of 20 cubic steps
f(x) = 1.5x - 0.5x^3 to max|err| = 1.3e-4 on x in [0, 1.08] (the singular
values after power-iteration scaling live in this interval). Since Newton-
Schulz iterations never rotate singular vectors, matching the scalar map
matches the matrix result.
"""
import base64
import contextlib
import math
import os
import sys

import numpy as np

for _p in ("/opt/trn_rl_repo", "/root/.axon_site/_ro/trn_rl_repo"):
    if _p not in sys.path and os.path.isdir(_p):
        sys.path.insert(0, _p)

import concourse.bacc as bacc
import concourse.bass as bass
import concourse.tile as tile
from concourse import mybir
from concourse.bass_utils import run_bass_kernel_spmd

F32 = mybir.dt.float32
F32R = mybir.dt.float32r
BF16 = mybir.dt.bfloat16
AF = mybir.ActivationFunctionType

NCORES = 8
B, C, H, W = 16, 256, 64, 64
BPC = B // NCORES            # batches per core
NUM_K = 5
USE_CC = os.environ.get("BCOP_USE_CC", "1") == "1"

# Fitted quintic schedule (see module docstring).
SCHED = [
    (+3.537126, -1.143386, +0.178025),
    (+3.486273, -1.233470, +0.114834),
    (+3.524281, -1.330306, +0.137033),
    (+3.425718, -1.406452, +0.147234),
    (+2.932084, -1.646435, +0.250418),
    (+2.298281, -1.697750, +0.420508),
    (+1.621210, -1.184795, +0.380696),
    (+2.043378, -1.577882, +0.524628),
]
NSTAGE = len(SCHED)

NWARM = int(os.environ.get("BCOP_NWARM", "14"))
PI_FILL = int(os.environ.get("BCOP_PI_FILL", "5"))
STAGE_FILL_A = int(os.environ.get("BCOP_SFILL_A", "3"))  # between G and M psum
STAGE_FILL_B = int(os.environ.get("BCOP_SFILL_B", "3"))  # between M psum and WV
GFILL_GROUPS = int(os.environ.get("BCOP_GFILL", "13"))

_U0_B64 = "/wEQPugDtb4BnP0/kWP6PuSbi7/gnwjA+PYNv7jlez+2hrk/+Y+PPtA6Vbxs3x4/+0hcv11wh7/Pq9O/NsCkviaW9j+UZw0/21Rvv94jl79SCki9/QoCPz9/ib+0vce/aajHvnf82z/Gv+o+JRSavyafvbyZUiE/loItv/YVRb/dh02/Wjd+vyiDq7/6VFO+LSBCPsANk77szwY9Ndkgv6E2Nj+02y4/fqQ6P0e+WT+IKJE/l2kHQBSnBT+tJJO/EEb/v5BoAr+4uIQ/izzlPwY70T4Ce7O/u9NDvvRCkT6XeQu9PR0iP+9lN7/ftDa/0QU5v5mfM78iwDK/Lx1Bv3MqTb8FaX+/1Buyv/xXL76+boI+W8ikvWE43z4HZrm/8s2bviaCfj3qVRO/uUh0PwmItT/MUpM+TIzqvANnKT9mokK/tXJLv0YvfL9LGKm/Wh1Pvv5NOD4f1J6+MwxsPQW9/L66eIg//yzRP0Biqz54rvO/iNcLvwLcaT8pZKg/RX3rPaDh6L4hGJk/ZIoIPFi3Gr/0TWI/4H2JP3c71T+K77Q+JJQUwLN9Kb+jgUA/7GpIP4RadT+Akbs/kkyfPqb0Ur2l3wA/KACEvzak5L8nBNS+tFauP1Xobz6lkE++Bj4/Ppgqmb5hyJs9lN0Jv4Tpbj8b0pg/Ci+fO15KHb98sU0/fGCBP8aBCkDeLwk/GuSIvwZLBMA+Yxq/PpFGP1M6OT8ET1Q/4a+JP8OhCUCqvh8/A9s6v4qOPL8CVT+/C8tFv4w5N78YZE+/Mz2Ev6ISJMDdigC/a5BqP7epwj+/XkM++G1WvoOHgz7vUom7PZMGPwEYd7/Ad6O/y/sEu+ENRb/dX1+/y+2Mv3qg+b9D3AG/s7uOP1+LIECyxyg/+j84v/l7ML/9azq/K9Rcv3xRhr/OQ82/OpewvhskDUCvGyg/pBk+v7AVWb88Do2/t9QRwAcJJb8SQjg/Q9osPzXNRD/KDkw/9zNzPyi4wT8SSoE+MQDvvVJh9T7k5KS/LnTpvdu47T7V7Ja/OoycvYveDD8crmS/Yvmhv487D74+xMc+K8jnvwRw376qtrY/48mVPtA+/7tHLB8/rT1Zvz7Wkb8dQgXAcV0Hv+JPjD/od0VAU8YcP4DyYr/sN4a/Y8rIv4B+y76lZd4/oiX3PrHrnr/3Bf29ogO1PjJID8AkLiO/IfY0Px4NMj/Nt0Y/H7RSP5oCbD/tS5M/vlQ6PZQAA79XcZE/bvcAQHj2+j6RSoi/evzOv2tqo741OwVAhOsEP5NEj78XKPG/0KgIvzMlbz8YjZo/Tx17PMARFr8SgU8/owtoP0azpT/0eMI9TjDTvmgMsD8wJyk+yFWEviUTHj73GL2+UHrOP9czpz4QGf+/Hbz7vggLiD/L3s4/lmSoPpyX/79Otvy+6L6HP4zLzT9b/qQ+ZpgGwGQ9Bb9SmY8/J9j0P5xdDT+J4mW/2Uahv/azFL6abcU+IUHpvzYp275a7cA/CFWFPo0eG76EjL4+h6jLv4bOyr4Oudw/1g7XPi/dlL+w5AW+oifVPkeclr9vPhO+2YryPjoWv78dBi++V88vPiz4LL4PIzE+gGkqvlixNj5Z9XG+UFOOPk0B7r2PzLo+YzcEwLiFG78IREQ/HvA1P4qzTD+TG5E/2hHOPwZA0z5HYZm/Ke+sveJpsT4BV+e/ADasvqqExz9uR50/UW+ePVq2/b7PWIw/+kD5Pwl1/D4eU42/Ucb0v8Ut/L7LIoQ/SJnZP/no+T6ZT6W/K4ndvQWz9j6sMKi/K6rFvVro4j71hMO/ubN3vgaesD1I7te+BliyP46aLz7WcoW+jFfxPUpyp76GG/k/oSoOP4Q7b7+l2pa/zYBsvUAdBT8BxpG/FpYIwCHGBL/5lZI/Z0//P83VAD8TXoG/eHDdvxl0+b5hVaE/+nwYPozOur6agtA/wJ2kPj6sAcB21f++/rqJPwHWzT+KSaw+H0v3v1EfFL+ECHk/t/K/PwJmdz717da9I9v0Po+snr8OhAe+3OClPjzABsBWFge/PPqKP7THDEC2eyE/LnQ4v2vpOL8DiDa/MP40v1KvMb8QQkW/Q+9Sv4cTcr/ZZJu/9ILMvMXHJj80AkS/WJBPv2zdab+Rnqa/JCDQvfLC9z5jzp2/cVMCvgN3rz4CjSzAt6QbvxN4YD9jPoc/ExHMP4/MzD4kN9e/PQzlvnoonT96EBQ9YxAjvzDiMz+Dei8/A9k+PxrlXz+RxIU/a8jEP3HJuD6fbNa/x5m3vt69DECJAiM/ZzQzvwMeLb9IFDu/F31Xv9LDjb8nfTbAVRgdv2f9Wz+EYIE/vpnWP5FR5D5255y/hK4PvY70Ij/JADS//IYuvw9JPb8x/1y/EraCv1Ddz7/p6NS+ssuWP6AvFj6Rt+++0frCPwj7Qj7hu1a+ffWAPu40N7zucgE/0R9mv3o9u7+cRm2+TrGUPrsEwr39A6Y+OpfQv0xi1r7LjJQ//CUHPg3b1b4R1ZY/MGkSPj948r6GYb8/IWYrPi3VML7i5CY+DaM8vrg8Zj6aZpm+BJ3dvvG/rz/5Xks+4/dNvqbITj6WTVu+k7lmPhPpJb7JiJ0+7Sonvc57AT9h9YW/hQ/Qv3EquL4P6iRA2vAfP2PRXL9i24e/spbLv8ksu76EVsk/bYfCPlP9078tJKy+uHAJQFAy/T4hMoW//m3Zv2o88L5uy54/Z3kNPvXhvL5oHsc/fSa6PpuUzr97Yau+Phz+P2grED8BsoC/VC2vv+umL76NWnc+E+nKvS5K7z4fT6O/bpvAvaz+0z7dbqm/WsJZvkHXKz4kGXO+zxbMPSVI774Sq6M/KJ+4PYvIzr5qNK8/H+YxPu+Sfr6UPqM9LszZvgsWvT/qp4I+DUHHvWY01T40Xa6/zj5qvmQnVj4xUiy+9Q6BPsrBw72x9c4+JjOyv3o1O75045k+Fp2UvW3DDT/1PWW/EGCiv1sCGr7qlsE+IGnFv8Dmub4Qw9Q/JvS1PokSDMAoHiO/WlE0PzdKLj/Qlj0/IOFePxFhhT/8N+s/CWDgPgi3ub9WIZm+QI+aPbZDC79IxWs/ua+VPwaKYD3RA/6+y52HP7H0yz84bMw+Iyvavz8J674naJY/LZeFPYobEL+tEX4/33+uPxePcj7SV0y+QXxFPt1Ckr71dsY8huspv7GZPz/Vv2E/sWmIPywH0D9HGqo+8ZP4v8dWEL+z8Xw/toysP1zFYz7yziu+DUs0PhJoI74P+0Q+wp5TvsIThD5CAKC9yJsgP+3VU793VYm/7pcFwFxFHL9RC0I/0ewvPwJWXD9KmWQ/JKu4P1z2XT6Hh3O+mm4MPRJZCr/TAoc/Co33Pxd1Jj9wUGG/8Ghwv0CJsL8kHoG+YMNoPHdIAb890Gc/6LO/P6rl3b4ma68/J/lTPs1wXL54S2k+hwFEvoYFiz4Ngd690izPPkdhrb/G41W+yX1EPovmob7N3Ig9tkULv2EvZT8OBKc/08C7PQCa3b7KSrs/Li2RPvovYLyykxs/WclVvz7vkr83F/W/9wARvwx5dj80zrk/KHeVPlqbqryZeCI/s/ctv5p2P79dQ16/byOCv8A34L8zk/S+ECWnP+ow6T2jrOy+yjSXP03Ccj2rwA6/TFl8PxMqrD/YO24+w8VUvp7yIj6rZYW+5C8SPh94yb56q9Y/yjDtPsf6lb/kv5a9UWAMP5QhZr8Jk52/xcEHvr2Xqj62NPO/xcsKvz8/cj8ar5k/uluEPLTnFL9eOVE/mwdtP8YZlz9W/5k9w6kIv/mdbz8xVJg/U1wbPPQXHb+v/14/F+iDPxUP5T8k2tg+WjCrv9HYWL4s/TM+niB4vmN5zT2uy/W+5nCiP7k1ID5fZ7q+huDTP6pDsT6oCynAZF4Zv+6xST96dXk/duXDP+O8jD4xhQK+3MOvPul3MsB0fhy/OSRcP2Imgj+fnts/edTtPsn8k79PFzy9ieMEP+YfkL+/5Pi/gdcPv1OVfT+8uq0/1tNsPtJ7Vr6qPDE+Y8x6vmLR0D3qh/W+axuiP7ezGz6F5b2+xsnMP2TqzT5Vr9a/MFTkvkrynD/tchI9Bicev38RPj8SwEI/xaMxP8bzXz8K4Gs/YPOqP+eOlT6Rdre9uUOrPt3zx7+nUuy+hxmrP6C7lj4tDqy9odayPkKu4r9fmKS+aKXTP5Dh3D6xhqW/3/iKvCpS/z561mu/VEerv5WOl74/B6I9wbS2vrfg2j+T68E+ZGPvv4GGIL/qzZw/C0KXPRrT+76rX4s/85H+PxV9ED+IEWa/k5qcvwCWgr26Wws/+dxjvzL3pL9lseu9j8LvPldhoL+TGwC+CCyoPqjr+r+QbAq/sCBoP/BooT+A8AU+2FeuvrCBDUDPnyc/crQ6vy3TYr/JKoO/1Ozmv5h33b6Y9Lk/Gx6WPjeZpLyWNyk/9TpEv7sUUr/kknC/rmyZv2xkeryl0hg/g45Mv+IVe7+v+6y/oP9ovpt+Sz7W+zm+zeecPv22mb3kgAw/+Jtlv2RLn7+MIwC+n7ezPomVCsDK5yS/ArgwP/7eQz+SGFQ/9TNrP3XNlz9b8nk9/WgOv5opfD/S7Kw/BU5wPoPZSb6Pe0Q+8GCWvs5LWzte7B+/+pdWP6zMjT9BZUZA+pQdP8azWL+9GJC/jkL7vwFaE7/+N3Q/UU62P0yekT6ewce8KxQsP0ulO78BQ1q/DjySv1NbBsBcdgS/lumPP9aJ+T98gBA/7Kt9v9jCrr+oMG++89tRPpKNNr7OtqE+ugNIvZUFBD9ylZG/x/UDwEeTAb+TWoI/3JrcP3Ro8T4i2aW/PZ/LvQYN+D6LHaC/5YAKvonDpD4T7QXAfIoEv1CdkD/+M/0/xE4TPzsfdb/iorq/+ZOdvmzecT1y8BO/E1pzP3kUtz9kzpI+e2SwvIl3Kz9IBDy//i9av4Uhg78VFhjAd9MSv0oljD/dKOE/69nNPhhSCsDNGAm/SfWIP7hvBEDecRo/BXFGv01QOb919FO/EeSIv1tLBcCpNhu/YgRFPw7yNT+Aj0w/heOQP7fHzj+3aNE+/nOav55oxL1sUaM+BW/Uv+9p4L7xsqA/Q4BxPcaRK7/Zxlc/pVhEvxXWXb8CZ42/aSP0v4gGBb+62oI/V2TVP/5TwT7MY92/STbnvlSIlT+UxIw9Y9wRv9NKfz+q+6o/o0RpPladb76k5mk+Ti5gvmZsTD4snSe+i6h9PhBl1r3Jhe4+ibibv42NmrwQWh0/77xev7quhr9t/sy/N26zvqfBJkDAXR4//IhVv8t1jb/rxBzAeiEYvwfqRj8RAng/WVvDP2p3hD7DUBa+cVvJPuLs2L+Vduq+eHaUP6HsUD1edfy+tD2HP91Sxj++LL4+MY7Ev1uSwb5bXMk/hyTNPqhO4b9lhfC+KnOlPxCBpz0GZd2+WpDDPworhD4I/w++jr/HPsII6b9c7t2+yCO/PxeDgj4cjrO975fcPuDsvL9zJXi+ZBrQPc9r+b6WOp8/CdkJPrNopb7uDQlAp50GPyVijb+ZA0/Ar2sev6KZWz8bRpE/4AkCQCyCAT+HqIK/gzHcv4FM8b769aY/yL3TPQTl8r4gJaU/LhHGPWUaz76DE7U/bjZHPn0wk74uQcg8rNAqvyq6PT/XO18/XeuEP5jN6T/Snts+o96/v46kgb7I87M9rsfZvoGowj/uj4k+/FAMvocTpD6yEgfAOMEFvwM1jj+Ww05Ag34fP0+5Vr9Vkoy/gdYdwKjaFb+TulA/PMFpPzRApz/WFt09RujuvlZbkz9j9hw9og4Jv9sfiT87hwVAJOIbP2peQ79otzK/v5BivxtPcb9tCrO/56yHvg9Chz0prSi/pW5eP0Y+aD/4yb4/UdkrPpn1Mr4/ICU+IwFDvmxFVj6CtIO+PPuEO0SNBr/wvHY/Cr+jP74mDjw3rwO/TcZ+P1TgmD9OZ7E9gtOtvr/ZED6WILS+hIf3PwXsAT9Zz5K/YsgRwPx5Fb8iTVg/4MqJP2aJzD9ozsI+5GbQv0aUtL4S/EdATLEXP/2HUb+L0WO/CGGov8R2p72NwdE+oYWsv1qFVr6LMz0+Y4iUvsuL6DyqnSS/nQYzP3T+Nj/w1TA/ggc/P/I5Vj9V9JI/k8n3PxaoEj8A2nK/Ph25v8Atnr73CTc9YnAHvxW9jT+S8RpAJwoYP+jIR79QyHe/LDzCv4/oib5qhvk9xEayvlcYREABFBw/E+lfvw8Fib9yic+/4cqlvqiCBUAnPQY/6luKv68XGsDj0ie/1cZBPw3jSD//M3U/8pa4P6jAjz5WW6i8rXoqP2mpP79muGC/mgOGvzqGx79z2r6+HHzJP3+sxz6AZua/5gHWvuCVrj93S2o+I01XvnBzMj64xXW+b9rpPbkr5r50K50/eKbyPDf8I7/hXzA/lllAP5A/SD+QpXQ/oVS6P7DKmz6JyYO9OfEPPy3ufL/bZqy/pJdfvsc6cD4nh1O+m/E5PiVNoL4jc1o9iEsBv+0OhD+Ij+Q/CzDUPt+Lrb8ZzGi+lmhcPvwiJb4JNok+IQcLvlNvpT7DhATAot8Cv1L9kj8g9ghAU30HP/p0i79aLxPAS00ov8TuQj+djEw/ohOAP5+gsT9aljI+GHx4vsK62z3wxO++b+aTP/EP/D1u/N2+oWyjP7uqXDsCqAa/AoB3P4ddpD+6dSY8FXICv/jCZD/3gLg/GTZZPqL9er6koLY8uFsSv34rjT9xqec//xitPpnXxb9ymOi+h0avP6m7dD4ApBa9DpoIP7jxiL8ZOgbAx3Ycvz1HQz8ubTE/d5dgPx1Kbj8="


def _u0():
    return np.frombuffer(base64.b64decode(_U0_B64), dtype="<f4").reshape(5, 256).copy()


def _mm256p(nc, ps512, terms):
    """[256,256] matmul sum into ONE [128,512] psum bank as a single
    accumulation group: result rows m*128..+128 at cols m*256..+256."""
    n = 0
    total = len(terms) * 4
    for m in range(2):
        for lhsT_tiles, rhs_tiles in terms:
            for kt in range(2):
                nc.tensor.matmul(
                    ps512[:, m * 256:(m + 1) * 256],
                    lhsT_tiles[kt][:, m * 128:(m + 1) * 128],
                    rhs_tiles[kt][:],
                    start=n == 0,
                    stop=n == total - 1,
                )
                n += 1


def _mm256(nc, psums, terms):
    """[256,256] matmul sum over terms: psums[m] += sum_p lhsT_p.T @ rhs_p."""
    for m in range(2):
        for pi, (lhsT_tiles, rhs_tiles) in enumerate(terms):
            for kt in range(2):
                nc.tensor.matmul(
                    psums[m][:],
                    lhsT_tiles[kt][:, m * 128:(m + 1) * 128],
                    rhs_tiles[kt][:],
                    start=pi == 0 and kt == 0,
                    stop=pi == len(terms) - 1 and kt == 1,
                )


def _mv256(nc, ps2, lhsT_tiles, z_tiles):
    """matvec into one [128,2] psum tile: half m lands in column m."""
    n = 0
    for m in range(2):
        for kt in range(2):
            nc.tensor.matmul(
                ps2[:, m:m + 1],
                lhsT_tiles[kt][:, m * 128:(m + 1) * 128],
                z_tiles[kt],
                start=n == 0,
                stop=n == 3,
            )
            n += 1


def build_nc(use_cc=USE_CC):
    nc = bacc.Bacc("TRN2", target_bir_lowering=False, debug=False,
                   num_devices=NCORES)

    n_mat_local = 1 if use_cc else NUM_K

    x_in = nc.dram_tensor("x", [BPC, C, H + 2, W + 2], BF16,
                           kind="ExternalInput")
    pm_in = nc.dram_tensor("pm", [n_mat_local, C, C], F32, kind="ExternalInput")
    pmT_in = nc.dram_tensor("pmT", [n_mat_local, C, C], F32, kind="ExternalInput")
    u0_in = nc.dram_tensor("u0", [n_mat_local, C, 1], F32, kind="ExternalInput")
    eye_in = nc.dram_tensor("eye1", [C, C], F32, kind="ExternalInput")
    eyec_in = nc.dram_tensor("eyecat", [128, 512], F32, kind="ExternalInput")
    bias_in = nc.dram_tensor("biasc", [C, 1], F32, kind="ExternalInput")
    out_dram = nc.dram_tensor("out", [BPC, C, H, W], F32, kind="ExternalOutput")

    with tile.TileContext(nc) as tc, contextlib.ExitStack() as top:
        const = top.enter_context(tc.tile_pool(name="const", bufs=1))
        xpool = top.enter_context(tc.tile_pool(name="xpool", bufs=1))
        tpool = top.enter_context(tc.tile_pool(name="tpool", bufs=1))
        v5pool = top.enter_context(tc.tile_pool(name="v5pool", bufs=1))
        vfin = top.enter_context(tc.tile_pool(name="vfin", bufs=1))

        eye = [const.tile([128, 256], F32, name=f"eye_{t}", tag=f"eye{t}")
               for t in range(2)]
        bias_c = [const.tile([128, 1], F32, name=f"bias_{t}", tag=f"bias{t}")
                  for t in range(2)]
        ones_f = const.tile([1, 128], F32, name="ones_f", tag="onesf")
        eyec = const.tile([128, 512], F32, name="eyec", tag="eyec")
        nc.sync.dma_start(eyec[:], eyec_in[:])
        for t in range(2):
            nc.sync.dma_start(eye[t][:], eye_in[t * 128:(t + 1) * 128, :])
            nc.sync.dma_start(bias_c[t][:], bias_in[t * 128:(t + 1) * 128, :])
        nc.any.memset(ones_f[:], 1.0)

        # ---- chain inputs staged first so their DMAs beat the big x DMAs ----
        chain_in = []
        cinp = top.enter_context(tc.tile_pool(name="cinp", bufs=1))
        for mi in range(n_mat_local):
            A = [cinp.tile([128, 256], F32, name=f"A_{mi}_{t}", tag=f"A{mi}{t}")
                 for t in range(2)]
            AT = [cinp.tile([128, 256], F32, name=f"AT_{mi}_{t}", tag=f"AT{mi}{t}")
                  for t in range(2)]
            z0 = [cinp.tile([128, 1], F32, name=f"z0_{mi}_{t}", tag=f"z0{mi}{t}")
                  for t in range(2)]
            for t in range(2):
                nc.sync.dma_start(A[t][:], pm_in[mi, t * 128:(t + 1) * 128, :])
                nc.sync.dma_start(AT[t][:], pmT_in[mi, t * 128:(t + 1) * 128, :])
                nc.sync.dma_start(z0[t][:], u0_in[mi, t * 128:(t + 1) * 128, :])
            chain_in.append((A, AT, z0))

        # per-stage scaled-identity lhsT for the b*G psum term:
        #   bI[i] = (b_i/sqrt(c_i)) * I128
        aeye0 = const.tile([128, 512], F32, name="aeye0", tag="aeye0")
        nc.scalar.mul(aeye0[:], eyec[:], float(SCHED[0][0]))
        bI = []
        for i, (a_i, b_i, c_i) in enumerate(SCHED):
            bi = const.tile([128, 128], F32R, name=f"bI_{i}", tag=f"bI{i}")
            nc.vector.tensor_scalar_mul(bi[:], eye[0][:, 0:128],
                                        float(b_i / math.sqrt(c_i)))
            bI.append(bi)

        # ---- early barrier: tiny AllGather aligns core start times so the
        # real gather's multi-phase mesh sees ~zero skew ---------------------
        barp = top.enter_context(tc.tile_pool(name="barp", bufs=1,
                                              space="DRAM"))
        if use_cc:
            # fire-and-forget: forces CC-ring/peer init to overlap the chain
            # so the real gather is the (fast) second collective
            bsrc = const.tile([128, 1], F32, name="bsrc", tag="bsrc")
            nc.vector.memset(bsrc[:], 1.0)
            bgin = barp.tile([128, 1], F32, name="bgin", tag="bgin")
            bgout = barp.tile([NCORES, 128, 1], F32, name="bgout", tag="bgout")
            nc.sync.dma_start(bgin[:], bsrc[:])
            nc.gpsimd.collective_compute(
                "AllGather", mybir.AluOpType.bypass,
                replica_groups=[list(range(NCORES))],
                ins=[bgin.opt()], outs=[bgout.opt()],
            )

        # ---- PE warmup burst (gated on the barrier) to lift the clock ------
        wsrc = const.tile([128, 512], F32, name="wsrc", tag="wsrc")
        nc.vector.memset(wsrc[:], 1.0)
        dummy2 = const.tile([128, 512], F32R, name="dummy2", tag="dummy2")
        nc.vector.tensor_copy(dummy2[:], wsrc[:])
        dummy_r = const.tile([128, 256], F32R, name="dummy_r", tag="dummyr")
        nc.scalar.copy(dummy_r[:], wsrc[:, 0:256])
        with tc.tile_pool(name="warmps", bufs=1, space="PSUM") as wps:
            wp_ = wps.tile([128, 512], F32, name="warm", tag="warm")
            for wi in range(NWARM):
                nc.tensor.matmul(wp_[:], dummy_r[:, 0:128], dummy2[:],
                                 start=wi == 0, stop=wi == NWARM - 1)

        # ---- x load: direct bf16 DMA into padded tile + tiny edge copies ----
        Xp = [[xpool.tile([128, 66, 66], BF16, name=f"xp_{b}_{g}",
                          tag=f"xp{b}{g}") for g in range(2)]
              for b in range(BPC)]
        for b in range(BPC):
            for g in range(2):
                nc.sync.dma_start(Xp[b][g][:],
                                  x_in[b, g * 128:(g + 1) * 128, :, :])

        # ---- per-matrix chain: power iteration + fitted quintic schedule ----
        def half(tile512):
            return [tile512[:, t * 256:(t + 1) * 256] for t in range(2)]

        def fill(tag_idx, n, free=256):
            for fi in range(n):
                fl = cps.tile([1, free], F32, name="fl", tag="fl0", bufs=1)
                nc.tensor.matmul(fl[:], dummy_r[0:128, 0:1],
                                 dummy2[:, 0:free], start=True, stop=True)

        def mmh(ps_half, m, lhsT_tiles, rhs_tiles, extra_bI=None):
            """Half-row product: ps_half = (lhsT^T rhs) rows m*128..+128,
            optionally + bI^T rhs_m as a 3rd matmul in the same group."""
            total = 2 + (1 if extra_bI is not None else 0)
            for kt in range(2):
                nc.tensor.matmul(ps_half[:],
                                 lhsT_tiles[kt][:, m * 128:(m + 1) * 128],
                                 rhs_tiles[kt][:],
                                 start=kt == 0, stop=kt == 1 and total == 2)
            if extra_bI is not None:
                nc.tensor.matmul(ps_half[:], extra_bI[:], rhs_tiles[m][:],
                                 start=False, stop=True)

        v_final = []
        with tc.tile_pool(name="chain", bufs=2) as cp, \
             tc.tile_pool(name="chps", bufs=2, space="PSUM") as cps:
            for mi in range(n_mat_local):
                pt = mi % 2
                A, AT, z0 = chain_in[mi]

                # fp32r copies of A / A^T (used by PI and as stage-0 W/V)
                Ar_sb = cp.tile([128, 512], F32R, name="Ar", tag=f"Ar{pt}",
                                bufs=1)
                ATr_sb = cp.tile([128, 512], F32R, name="ATr", tag=f"ATr{pt}",
                                 bufs=1)
                for t in range(2):
                    nc.vector.tensor_copy(Ar_sb[:, t * 256:(t + 1) * 256],
                                          A[t][:])
                    nc.scalar.copy(ATr_sb[:, t * 256:(t + 1) * 256], AT[t][:])
                Ar, ATr = half(Ar_sb), half(ATr_sb)

                # Deferred-normalization power iteration (replicates the
                # reference's 10 normalized PI rounds up to fp noise):
                # G2 = A A^T, G4 = G2^2, G8 = G4^2; z9 = G8 G2 z0;
                # y = A^T z9; zf = A y; s = sqrt(|zf|^2/|y|^2).
                # Stage-0's unscaled G-hat = A^T A is interleaved here; the
                # 1/s^2 and 1/s factors fold into stage-0's copies.
                g2h = [cps.tile([128, 256], F32, name=f"g2_{m}",
                                tag=("hA", "hB")[m]) for m in range(2)]
                for m in range(2):
                    mmh(g2h[m], m, AT, AT)
                G2_sb = cp.tile([128, 512], F32, name="G2_sb", tag=f"G2{pt}",
                                bufs=1)
                nc.scalar.copy(G2_sb[:, 0:256], g2h[0][:])
                nc.vector.tensor_copy(G2_sb[:, 256:512], g2h[1][:])
                G2 = half(G2_sb)
                # stage-0 G-hat (independent of the scale chain)
                gph = [cps.tile([128, 256], F32, name=f"gp_{m}",
                                tag=("hC", "hD")[m], bufs=1) for m in range(2)]
                for m in range(2):
                    mmh(gph[m], m, Ar, Ar)
                g4h = [cps.tile([128, 256], F32, name=f"g4_{m}",
                                tag=("hA", "hB")[m]) for m in range(2)]
                for m in range(2):
                    mmh(g4h[m], m, G2, G2)
                Gu_sb = cp.tile([128, 512], F32R, name="Gu_sb",
                                tag=f"Gu{pt}", bufs=1)
                nc.scalar.copy(Gu_sb[:, 0:256], gph[0][:])
                nc.vector.tensor_copy(Gu_sb[:, 256:512], gph[1][:])
                Gu = half(Gu_sb)
                G4_sb = cp.tile([128, 512], F32, name="G4_sb", tag=f"G4{pt}",
                                bufs=1)
                nc.scalar.copy(G4_sb[:, 0:256], g4h[0][:])
                nc.vector.tensor_copy(G4_sb[:, 256:512], g4h[1][:])
                G4 = half(G4_sb)
                # z1 = G2 z0 while G8 is on the PE
                zp = cps.tile([128, 2], F32, name="zp", tag="pv0", bufs=1)
                _mv256(nc, zp, G2, z0)
                zs = cp.tile([128, 2], F32, name="zs", tag=f"z{pt}")
                nc.vector.tensor_copy(zs[:], zp[:])
                z = [zs[:, 0:1], zs[:, 1:2]]
                g8h = [cps.tile([128, 256], F32, name=f"g8_{m}",
                                tag=("hA", "hB")[m]) for m in range(2)]
                for m in range(2):
                    mmh(g8h[m], m, G4, G4)
                G8_sb = cp.tile([128, 512], F32, name="G8_sb", tag=f"G8{pt}",
                                bufs=1)
                nc.scalar.copy(G8_sb[:, 0:256], g8h[0][:])
                nc.vector.tensor_copy(G8_sb[:, 256:512], g8h[1][:])
                G8 = half(G8_sb)
                g2u = [cps.tile([128, 256], F32, name=f"g2u_{m}",
                                tag=("hA", "hB")[m]) for m in range(2)]
                for m in range(2):
                    mmh(g2u[m], m, Gu, Gu)
                fill(0, PI_FILL)
                # z9 = G8 z1 ; y = A^T z9 ; zf = A y
                zp = cps.tile([128, 2], F32, name="zp9", tag="pv0", bufs=1)
                _mv256(nc, zp, G8, z)
                zs = cp.tile([128, 2], F32, name="zs9", tag=f"z9{pt}")
                nc.vector.tensor_copy(zs[:], zp[:])
                z = [zs[:, 0:1], zs[:, 1:2]]
                fill(1, 2)
                yp = cps.tile([128, 2], F32, name="yp", tag="pv0", bufs=1)
                _mv256(nc, yp, A, z)
                ys = cp.tile([128, 2], F32, name="ys", tag=f"y{pt}")
                nc.vector.tensor_copy(ys[:], yp[:])
                y = [ys[:, 0:1], ys[:, 1:2]]
                fill(0, 2)
                zp = cps.tile([128, 2], F32, name="zfp", tag="pv0", bufs=1)
                _mv256(nc, zp, AT, y)
                zs = cp.tile([128, 2], F32, name="zfs", tag=f"zf{pt}")
                nc.vector.tensor_copy(zs[:], zp[:])
                z = [zs[:, 0:1], zs[:, 1:2]]
                fill(1, 2)

                # ny = y.y ; nz = z.z ; inv_s = sqrt(ny/nz) bcast;
                # inv_s2 = ny/nz bcast
                nyp = cps.tile([128, 1], F32, name="nyp", tag="pv0", bufs=1)
                for t in range(2):
                    nc.tensor.matmul(nyp[0:1, 0:1], y[t], y[t],
                                     start=t == 0, stop=t == 1)
                nzp = cps.tile([128, 1], F32, name="nzp", tag="pv0", bufs=1)
                for t in range(2):
                    nc.tensor.matmul(nzp[0:1, 0:1], z[t], z[t],
                                     start=t == 0, stop=t == 1)
                sc = cp.tile([1, 6], F32, name="sc", tag=f"sc{pt}")
                nc.vector.tensor_copy(sc[:, 0:1], nyp[0:1, :])
                nc.vector.reciprocal(sc[:, 1:2], nzp[0:1, :])
                nc.vector.tensor_scalar_mul(sc[:, 2:3], sc[:, 0:1],
                                            sc[:, 1:2])
                nc.scalar.sqrt(sc[:, 3:4], sc[:, 2:3])
                nc.vector.tensor_scalar_mul(sc[:, 4:5], sc[:, 2:3],
                                            float(SCHED[0][1]))
                nc.vector.tensor_scalar(sc[:, 5:6], sc[:, 2:3], sc[:, 2:3],
                                        float(SCHED[0][2]),
                                        op0=mybir.AluOpType.mult,
                                        op1=mybir.AluOpType.mult)
                bcp = cps.tile([128, 3], F32, name="bcp", tag="pv0", bufs=1)
                nc.tensor.matmul(bcp[:], ones_f[:], sc[:, 3:6],
                                 start=True, stop=True)
                inv_b = cp.tile([128, 3], F32, name="inv_b", tag=f"invb{pt}")
                nc.scalar.copy(inv_b[:], bcp[:])
                inv_s = inv_b[:, 0:1]    # 1/s broadcast
                b0_bc = inv_b[:, 1:2]    # b_0/s^2 broadcast
                c0_bc = inv_b[:, 2:3]    # c_0/s^4 broadcast

                w, v = Ar, ATr
                for it in range(NSTAGE):
                    a_i, b_i, c_i = SCHED[it]
                    sq_i = float(math.sqrt(c_i))
                    last = it == NSTAGE - 1
                    if it == 0:
                        gph_it = gph   # precomputed A^T A (unscaled)
                    else:
                        gph_it = [cps.tile([128, 256], F32, name=f"gp_{m}",
                                           tag=("hC", "hD")[m], bufs=1)
                                  for m in range(2)]
                        for m in range(2):
                            mmh(gph_it[m], m, w, w)
                    M_sb = cp.tile([128, 512], F32R, name="M_sb", tag=f"M{pt}")
                    if it == 0:
                        # M = a0*I + (b0/s^2)*Ghat + (c0/s^4)*Ghat^2, built
                        # straight from the unscaled psums with runtime bcasts
                        u1 = cp.tile([128, 512], F32, name="u1",
                                     tag=f"u1{pt}", bufs=1)
                        for m in range(2):
                            h = slice(m * 256, (m + 1) * 256)
                            nc.vector.scalar_tensor_tensor(
                                u1[:, h], gph_it[m][:], b0_bc, aeye0[:, h],
                                op0=mybir.AluOpType.mult,
                                op1=mybir.AluOpType.add)
                            nc.vector.scalar_tensor_tensor(
                                M_sb[:, h], g2u[m][:], c0_bc, u1[:, h],
                                op0=mybir.AluOpType.mult,
                                op1=mybir.AluOpType.add)
                    else:
                        Gs_sb = cp.tile([128, 512], F32R, name="Gs_sb",
                                        tag=f"G{pt}")
                        nc.scalar.mul(Gs_sb[:, 0:256], gph_it[0][:], sq_i)
                        nc.vector.tensor_scalar_mul(Gs_sb[:, 256:512],
                                                    gph_it[1][:], sq_i)
                        Gs = half(Gs_sb)
                        fill(it, STAGE_FILL_A)
                        # mp = c*G^2 + b*G per half (separate groups)
                        mph = [cps.tile([128, 256], F32, name=f"mp_{m}",
                                        tag=("hA", "hB")[m]) for m in range(2)]
                        for m in range(2):
                            mmh(mph[m], m, Gs, Gs, extra_bI=bI[it])
                        # M = a_i*eyec + mp per half
                        nc.vector.scalar_tensor_tensor(
                            M_sb[:, 0:256], eyec[:, 0:256], float(a_i),
                            mph[0][:],
                            op0=mybir.AluOpType.mult, op1=mybir.AluOpType.add)
                        nc.vector.scalar_tensor_tensor(
                            M_sb[:, 256:512], eyec[:, 256:512], float(a_i),
                            mph[1][:],
                            op0=mybir.AluOpType.mult, op1=mybir.AluOpType.add)
                    M = half(M_sb)
                    fill(it + 1, STAGE_FILL_B)
                    if last and use_cc:
                        # Only V'[0:128,:] feeds the gather.
                        vph = cps.tile([128, 256], F32, name="vp2", tag="hC",
                                       bufs=1)
                        for kt in range(2):
                            nc.tensor.matmul(vph[:],
                                             M[kt][:, 0:128], v[kt],
                                             start=kt == 0, stop=kt == 1)
                        v_sb = vfin.tile([128, 512], F32R, name=f"vf_{mi}",
                                         tag=f"vf{mi}")
                        nc.vector.tensor_copy(v_sb[:, 0:256], vph[:])
                        v = half(v_sb)
                        v_final.append(v)
                        break
                    # W' = W M (lhsT = V), V' = M V (lhsT = M), per half
                    wph = [cps.tile([128, 256], F32, name=f"wp_{m}",
                                    tag=("hA", "hB")[m]) for m in range(2)]
                    vph = [cps.tile([128, 256], F32, name=f"vp_{m}",
                                    tag=("hC", "hD")[m], bufs=1) for m in range(2)]
                    for m in range(2):
                        mmh(wph[m], m, v, M)
                    for m in range(2):
                        mmh(vph[m], m, M, v)
                    fill(it, 2)
                    pool = vfin if last else cp
                    w_sb = cp.tile([128, 512], F32R, name="w_sb", tag=f"w{pt}")
                    v_sb = pool.tile([128, 512], F32R,
                                     name=f"vf_{mi}" if last else "v_sb",
                                     tag=f"vf{mi}" if last else f"v{pt}")
                    if it == 0:
                        nc.scalar.mul(w_sb[:, 0:256], wph[0][:], inv_s)
                        nc.vector.tensor_scalar_mul(w_sb[:, 256:512],
                                                    wph[1][:], inv_s)
                        nc.scalar.mul(v_sb[:, 0:256], vph[0][:], inv_s)
                        nc.vector.tensor_scalar_mul(v_sb[:, 256:512],
                                                    vph[1][:], inv_s)
                    else:
                        nc.scalar.copy(w_sb[:, 0:256], wph[0][:])
                        nc.vector.tensor_copy(w_sb[:, 256:512], wph[1][:])
                        nc.scalar.copy(v_sb[:, 0:256], vph[0][:])
                        nc.vector.tensor_copy(v_sb[:, 256:512], vph[1][:])
                    w, v = half(w_sb), half(v_sb)
                else:
                    v_final.append(v)

        # ---- gather the needed V halves across cores ------------------------
        # Rank r carries matrix [1,2,3,4, 0, 0*Q, 1, 2][r], where Q swaps
        # column halves - ortho(A Q) = ortho(A) Q, so rank 5's V[:128]
        # equals V_0[128:256].
        V0 = [None, None]
        Vh = [None] * 4
        if use_cc:
            with tc.tile_pool(name="ccdram", bufs=1, space="DRAM") as dp, \
                 tc.tile_pool(name="vstg", bufs=4) as vstg, \
                 tc.tile_pool(name="fillps", bufs=2, space="PSUM") as fps:
                gin = dp.tile([128, C], BF16, name="gin", tag="gin")
                gout = dp.tile([NCORES, 128, C], BF16, name="gout", tag="gout")
                vbf = vstg.tile([128, C], BF16, name="vbf", tag="vbf")
                nc.vector.tensor_copy(vbf[:], v_final[0][0])
                nc.sync.dma_start(gin[:, :], vbf[:])
                gate = vstg.tile([128, 1], F32R, name="gate", tag="gate")
                nc.vector.tensor_copy(gate[:], v_final[0][0][:, 0:1])
                nc.gpsimd.collective_compute(
                    "AllGather", mybir.AluOpType.bypass,
                    replica_groups=[list(range(NCORES))],
                    ins=[gin.opt()], outs=[gout.opt()],
                )
                for gi in range(GFILL_GROUPS):
                    fl = fps.tile([1, 512], F32, name=f"gfill_{gi}",
                                  tag=f"gfill{gi % 2}")
                    for wi in range(6):
                        nc.tensor.matmul(fl[:], gate[:, 0:1], dummy2[:],
                                         start=wi == 0, stop=wi == 5)
                for slot, dest in [(0, ("vh", 0)), (1, ("vh", 1)),
                                   (2, ("vh", 2)), (3, ("vh", 3)),
                                   (4, ("v0", 0)), (5, ("v0", 1))]:
                    kind, idx = dest
                    vs = v5pool.tile([128, 256], BF16, name=f"{kind}_{idx}",
                                     tag=f"{kind}{idx}")
                    nc.sync.dma_start(vs[:], gout[slot, :, :])
                    if kind == "vh":
                        Vh[idx] = vs
                    else:
                        V0[idx] = vs
        else:
            for t in range(2):
                v0 = v5pool.tile([128, 256], BF16, name=f"v50_{t}", tag=f"v50{t}")
                nc.vector.tensor_copy(v0[:], v_final[0][t])
                V0[t] = v0
            for b in range(4):
                vh = v5pool.tile([128, 256], BF16, name=f"vh_{b}", tag=f"vh{b}")
                nc.vector.tensor_copy(vh[:], v_final[1 + b][0])
                Vh[b] = vh

        # ---- tail: PQ, block_orth pair products, matrix_conv, T -------------
        Ttap = [[[tpool.tile([128, 256], BF16, name=f"T_{k}_{l}_{t}",
                             tag=f"T{k}{l}{t}")
                  for t in range(2)] for l in range(3)] for k in range(3)]
        with tc.tile_pool(name="tail", bufs=1) as tl, \
             tc.tile_pool(name="tailps", bufs=1, space="PSUM") as tps:
            if use_cc:
                for wi in range(2):
                    twf = tps.tile([1, 256], F32, name=f"twf_{wi}",
                                   tag="twf0")
                    nc.tensor.matmul(twf[:], Vh[0][:, 0:1], Vh[0][:],
                                     start=True, stop=True)
            PQ = []
            for b in range(4):
                pq = [tl.tile([128, 256], F32R, name=f"pq_{b}_{t}",
                              tag=f"pq{b}{t}") for t in range(2)]
                ps = tps.tile([128, 512], F32, name="pqps", tag="pqps")
                for m in range(2):
                    nc.tensor.matmul(ps[:, m * 256:(m + 1) * 256],
                                     Vh[b][:, m * 128:(m + 1) * 128],
                                     Vh[b][:],
                                     start=True, stop=True)
                for m in range(2):
                    nc.scalar.copy(pq[m][:], ps[:, m * 256:(m + 1) * 256])
                del ps
                PQ.append(pq)

            def pair_products(pa, pb, name):
                """e[c][r]: [0][0]=pa@pb, [0][1]=pa-C, [1][0]=pb-C,
                [1][1]=I-pa-pb+C (symmetric projection algebra)."""
                ps = [tps.tile([128, 256], F32, name=f"ccps_{t}", tag=f"ccps{t}")
                      for t in range(2)]
                _mm256(nc, ps, [(pa, pb)])
                e = [[[tl.tile([128, 256], F32R, name=f"{name}_e{i}{j}_{t}",
                               tag=f"{name}e{i}{j}{t}")
                       for t in range(2)] for j in range(2)] for i in range(2)]
                q = [tl.tile([128, 256], F32, name=f"{name}_q_{t}",
                             tag=f"{name}q{t}") for t in range(2)]
                for t in range(2):
                    nc.scalar.copy(e[0][0][t][:], ps[t][:])
                    nc.vector.tensor_sub(e[0][1][t][:], pa[t][:],
                                         e[0][0][t][:].bitcast(F32))
                    nc.vector.tensor_sub(e[1][0][t][:], pb[t][:],
                                         e[0][0][t][:].bitcast(F32))
                    nc.vector.tensor_sub(q[t][:], eye[t][:],
                                         pa[t][:].bitcast(F32))
                    nc.vector.tensor_sub(e[1][1][t][:], q[t][:],
                                         e[1][0][t][:].bitcast(F32))
                return e

            # m1T[c1][r1] = a2[c1] a1[r1]; m2[r2][c2] = a3[r2] a4[c2]
            m1T = pair_products(PQ[1], PQ[0], "m1T")
            m2 = pair_products(PQ[2], PQ[3], "m2")

            with tc.tile_pool(name="p3pool", bufs=3) as p3p:
                for i in range(3):
                    for j in range(3):
                        terms = [(i1, j1) for i1 in range(min(2, i + 1))
                                 for j1 in range(min(2, j + 1))
                                 if i - i1 < 2 and j - j1 < 2]
                        ps = [tps.tile([128, 256], F32, name=f"p3ps_{t}",
                                       tag=f"p3ps{t}") for t in range(2)]
                        _mm256(nc, ps, [(m1T[j1][i1], m2[i - i1][j - j1])
                                        for (i1, j1) in terms])
                        cell = [p3p.tile([128, 256], BF16, name=f"cell_{t}",
                                         tag=f"cell{t}") for t in range(2)]
                        for t in range(2):
                            nc.scalar.copy(cell[t][:], ps[t][:])
                        tp = [tps.tile([128, 256], F32, name=f"tps_t{t}",
                                       tag=f"tpsT{t}") for t in range(2)]
                        _mm256(nc, tp, [(V0, cell)])
                        for t in range(2):
                            nc.scalar.copy(Ttap[i][j][t][:], tp[t][:])

        # ---- conv: out[o, pix] += T[kw][kh][i, o] * Xp[i, pix+tap] ----------
        with tc.tile_pool(name="ops", bufs=8, space="PSUM") as ops, \
             tc.tile_pool(name="ostg", bufs=8) as ostg:
            for b in range(BPC):
                for ot in range(2):
                    for q in range(4):
                        ptiles = [ops.tile([128, 512], F32, name=f"cps_{k}",
                                           tag="convps") for k in range(2)]
                        first, last = (0, 0), (8, 1)
                        for tap in range(9):
                            kh, kw = tap // 3, tap % 3
                            for kt in range(2):
                                lhs = Ttap[kw][kh][kt][:, ot * 128:(ot + 1) * 128]
                                for k in range(2):
                                    h0 = q * 16 + k * 8
                                    rhs = Xp[b][kt][:, h0 + kh:h0 + kh + 8,
                                                    kw:kw + 64]
                                    nc.tensor.matmul(
                                        ptiles[k][:], lhs, rhs,
                                        start=(tap, kt) == first,
                                        stop=(tap, kt) == last)
                        for k in range(2):
                            h0 = q * 16 + k * 8
                            so = ostg.tile([128, 512], F32, name="so",
                                           tag="ostg")
                            if k == 0:
                                nc.scalar.activation(
                                    so[:], ptiles[k][:], AF.Identity,
                                    bias=bias_c[ot][:], scale=1.0)
                            else:
                                nc.vector.tensor_scalar_add(
                                    so[:], ptiles[k][:], bias_c[ot][:])
                            nc.sync.dma_start(
                                out_dram[b, ot * 128:(ot + 1) * 128,
                                         h0:h0 + 8, :].rearrange(
                                             "c h w -> c (h w)"),
                                so[:])

    nc.compile()
    return nc


_CACHE = {}


def _get_nc():
    key = (USE_CC,)
    if key not in _CACHE:
        _CACHE[key] = build_nc(USE_CC)
    return _CACHE[key]


def make_in_maps(x, param_matrices, bias, use_cc=None):
    if use_cc is None:
        use_cc = USE_CC
    import ml_dtypes
    x = np.ascontiguousarray(x, dtype=np.float32).astype(ml_dtypes.bfloat16)
    x = np.pad(x, ((0, 0), (0, 0), (1, 1), (1, 1)), mode="wrap")
    pm = np.ascontiguousarray(param_matrices, dtype=np.float32)
    bias = np.ascontiguousarray(bias, dtype=np.float32)
    u0 = _u0()
    eye1 = np.eye(C, dtype=np.float32)
    eyecat = np.zeros((128, 512), np.float32)
    for t in range(2):
        eyecat[:, t * 256 + t * 128:t * 256 + t * 128 + 128] = np.eye(128)
    bias_c = bias.reshape(C, 1)
    pmT = np.ascontiguousarray(pm.transpose(0, 2, 1))
    in_maps = []
    perm = np.concatenate([np.arange(128, 256), np.arange(128)])
    pm0q = np.ascontiguousarray(pm[0][:, perm])
    pm0qT = np.ascontiguousarray(pm0q.T)
    sel_map = [1, 2, 3, 4, 0, -1, 1, 2]   # -1 = matrix 0 * Q
    for c in range(NCORES):
        if use_cc:
            sel = sel_map[c]
            if sel == -1:
                pm_l = pm0q[None]
                pmT_l = pm0qT[None]
                u0_l = u0[0:1, :, None]
            else:
                pm_l = pm[sel:sel + 1]
                pmT_l = pmT[sel:sel + 1]
                u0_l = u0[sel:sel + 1, :, None]
        else:
            pm_l, pmT_l, u0_l = pm, pmT, u0[:, :, None]
        in_maps.append({
            "x": x[c * BPC:(c + 1) * BPC],
            "pm": np.ascontiguousarray(pm_l),
            "pmT": np.ascontiguousarray(pmT_l),
            "u0": np.ascontiguousarray(u0_l),
            "eye1": eye1,
            "eyecat": eyecat,
            "biasc": bias_c,
        })
    return in_maps


def kernel(x, param_matrices, bias, _trace=False):
    nc = _get_nc()
    in_maps = make_in_maps(x, param_matrices, bias)
    res = run_bass_kernel_spmd(nc, in_maps, list(range(NCORES)), trace=_trace)
    out = np.concatenate([res.results[c]["out"] for c in range(NCORES)], axis=0)
    if _trace:
        kernel._last_result = res
    return out


# revision 42
# speedup vs baseline: 1.2765x; 1.0006x over previous
"""Trainium2 Bass kernel for nn_BCOP (Bjorck-orthonormalized circular conv).

Self-contained: builds weights (power iteration + fitted 8-stage quintic
Newton-Schulz schedule matching the reference's 20 cubic Bjorck iterations)
and the 3x3 circular conv on 8 NeuronCores, data-parallel over batch, with
the per-matrix chains distributed over cores 0-5 and AllGathered.

The 8-stage quintic schedule q_i(x) = a_i x + b_i x^3 + c_i x^5 was fitted
offline so the composition matches the composition of 20 cubic steps
f(x) = 1.5x - 0.5x^3 to max|err| = 1.3e-4 on x in [0, 1.08] (the singular
values after power-iteration scaling live in this interval). Since Newton-
Schulz iterations never rotate singular vectors, matching the scalar map
matches the matrix result.
"""
import base64
import contextlib
import math
import os
import sys

import numpy as np

for _p in ("/opt/trn_rl_repo", "/root/.axon_site/_ro/trn_rl_repo"):
    if _p not in sys.path and os.path.isdir(_p):
        sys.path.insert(0, _p)

import concourse.bacc as bacc
import concourse.bass as bass
import concourse.tile as tile
from concourse import mybir
from concourse.bass_utils import run_bass_kernel_spmd

F32 = mybir.dt.float32
F32R = mybir.dt.float32r
BF16 = mybir.dt.bfloat16
AF = mybir.ActivationFunctionType

NCORES = 8
B, C, H, W = 16, 256, 64, 64
BPC = B // NCORES            # batches per core
NUM_K = 5
USE_CC = os.environ.get("BCOP_USE_CC", "1") == "1"

# Fitted quintic schedule (see module docstring).
SCHED = [
    (+3.537126, -1.143386, +0.178025),
    (+3.486273, -1.233470, +0.114834),
    (+3.524281, -1.330306, +0.137033),
    (+3.425718, -1.406452, +0.147234),
    (+2.932084, -1.646435, +0.250418),
    (+2.298281, -1.697750, +0.420508),
    (+1.621210, -1.184795, +0.380696),
    (+2.043378, -1.577882, +0.524628),
]
NSTAGE = len(SCHED)

NWARM = int(os.environ.get("BCOP_NWARM", "14"))
PI_FILL = int(os.environ.get("BCOP_PI_FILL", "3"))
STAGE_FILL_A = int(os.environ.get("BCOP_SFILL_A", "3"))  # between G and M psum
STAGE_FILL_B = int(os.environ.get("BCOP_SFILL_B", "3"))  # between M psum and WV
GFILL_GROUPS = int(os.environ.get("BCOP_GFILL", "13"))

_U0_B64 = "/wEQPugDtb4BnP0/kWP6PuSbi7/gnwjA+PYNv7jlez+2hrk/+Y+PPtA6Vbxs3x4/+0hcv11wh7/Pq9O/NsCkviaW9j+UZw0/21Rvv94jl79SCki9/QoCPz9/ib+0vce/aajHvnf82z/Gv+o+JRSavyafvbyZUiE/loItv/YVRb/dh02/Wjd+vyiDq7/6VFO+LSBCPsANk77szwY9Ndkgv6E2Nj+02y4/fqQ6P0e+WT+IKJE/l2kHQBSnBT+tJJO/EEb/v5BoAr+4uIQ/izzlPwY70T4Ce7O/u9NDvvRCkT6XeQu9PR0iP+9lN7/ftDa/0QU5v5mfM78iwDK/Lx1Bv3MqTb8FaX+/1Buyv/xXL76+boI+W8ikvWE43z4HZrm/8s2bviaCfj3qVRO/uUh0PwmItT/MUpM+TIzqvANnKT9mokK/tXJLv0YvfL9LGKm/Wh1Pvv5NOD4f1J6+MwxsPQW9/L66eIg//yzRP0Biqz54rvO/iNcLvwLcaT8pZKg/RX3rPaDh6L4hGJk/ZIoIPFi3Gr/0TWI/4H2JP3c71T+K77Q+JJQUwLN9Kb+jgUA/7GpIP4RadT+Akbs/kkyfPqb0Ur2l3wA/KACEvzak5L8nBNS+tFauP1Xobz6lkE++Bj4/Ppgqmb5hyJs9lN0Jv4Tpbj8b0pg/Ci+fO15KHb98sU0/fGCBP8aBCkDeLwk/GuSIvwZLBMA+Yxq/PpFGP1M6OT8ET1Q/4a+JP8OhCUCqvh8/A9s6v4qOPL8CVT+/C8tFv4w5N78YZE+/Mz2Ev6ISJMDdigC/a5BqP7epwj+/XkM++G1WvoOHgz7vUom7PZMGPwEYd7/Ad6O/y/sEu+ENRb/dX1+/y+2Mv3qg+b9D3AG/s7uOP1+LIECyxyg/+j84v/l7ML/9azq/K9Rcv3xRhr/OQ82/OpewvhskDUCvGyg/pBk+v7AVWb88Do2/t9QRwAcJJb8SQjg/Q9osPzXNRD/KDkw/9zNzPyi4wT8SSoE+MQDvvVJh9T7k5KS/LnTpvdu47T7V7Ja/OoycvYveDD8crmS/Yvmhv487D74+xMc+K8jnvwRw376qtrY/48mVPtA+/7tHLB8/rT1Zvz7Wkb8dQgXAcV0Hv+JPjD/od0VAU8YcP4DyYr/sN4a/Y8rIv4B+y76lZd4/oiX3PrHrnr/3Bf29ogO1PjJID8AkLiO/IfY0Px4NMj/Nt0Y/H7RSP5oCbD/tS5M/vlQ6PZQAA79XcZE/bvcAQHj2+j6RSoi/evzOv2tqo741OwVAhOsEP5NEj78XKPG/0KgIvzMlbz8YjZo/Tx17PMARFr8SgU8/owtoP0azpT/0eMI9TjDTvmgMsD8wJyk+yFWEviUTHj73GL2+UHrOP9czpz4QGf+/Hbz7vggLiD/L3s4/lmSoPpyX/79Otvy+6L6HP4zLzT9b/qQ+ZpgGwGQ9Bb9SmY8/J9j0P5xdDT+J4mW/2Uahv/azFL6abcU+IUHpvzYp275a7cA/CFWFPo0eG76EjL4+h6jLv4bOyr4Oudw/1g7XPi/dlL+w5AW+oifVPkeclr9vPhO+2YryPjoWv78dBi++V88vPiz4LL4PIzE+gGkqvlixNj5Z9XG+UFOOPk0B7r2PzLo+YzcEwLiFG78IREQ/HvA1P4qzTD+TG5E/2hHOPwZA0z5HYZm/Ke+sveJpsT4BV+e/ADasvqqExz9uR50/UW+ePVq2/b7PWIw/+kD5Pwl1/D4eU42/Ucb0v8Ut/L7LIoQ/SJnZP/no+T6ZT6W/K4ndvQWz9j6sMKi/K6rFvVro4j71hMO/ubN3vgaesD1I7te+BliyP46aLz7WcoW+jFfxPUpyp76GG/k/oSoOP4Q7b7+l2pa/zYBsvUAdBT8BxpG/FpYIwCHGBL/5lZI/Z0//P83VAD8TXoG/eHDdvxl0+b5hVaE/+nwYPozOur6agtA/wJ2kPj6sAcB21f++/rqJPwHWzT+KSaw+H0v3v1EfFL+ECHk/t/K/PwJmdz717da9I9v0Po+snr8OhAe+3OClPjzABsBWFge/PPqKP7THDEC2eyE/LnQ4v2vpOL8DiDa/MP40v1KvMb8QQkW/Q+9Sv4cTcr/ZZJu/9ILMvMXHJj80AkS/WJBPv2zdab+Rnqa/JCDQvfLC9z5jzp2/cVMCvgN3rz4CjSzAt6QbvxN4YD9jPoc/ExHMP4/MzD4kN9e/PQzlvnoonT96EBQ9YxAjvzDiMz+Dei8/A9k+PxrlXz+RxIU/a8jEP3HJuD6fbNa/x5m3vt69DECJAiM/ZzQzvwMeLb9IFDu/F31Xv9LDjb8nfTbAVRgdv2f9Wz+EYIE/vpnWP5FR5D5255y/hK4PvY70Ij/JADS//IYuvw9JPb8x/1y/EraCv1Ddz7/p6NS+ssuWP6AvFj6Rt+++0frCPwj7Qj7hu1a+ffWAPu40N7zucgE/0R9mv3o9u7+cRm2+TrGUPrsEwr39A6Y+OpfQv0xi1r7LjJQ//CUHPg3b1b4R1ZY/MGkSPj948r6GYb8/IWYrPi3VML7i5CY+DaM8vrg8Zj6aZpm+BJ3dvvG/rz/5Xks+4/dNvqbITj6WTVu+k7lmPhPpJb7JiJ0+7Sonvc57AT9h9YW/hQ/Qv3EquL4P6iRA2vAfP2PRXL9i24e/spbLv8ksu76EVsk/bYfCPlP9078tJKy+uHAJQFAy/T4hMoW//m3Zv2o88L5uy54/Z3kNPvXhvL5oHsc/fSa6PpuUzr97Yau+Phz+P2grED8BsoC/VC2vv+umL76NWnc+E+nKvS5K7z4fT6O/bpvAvaz+0z7dbqm/WsJZvkHXKz4kGXO+zxbMPSVI774Sq6M/KJ+4PYvIzr5qNK8/H+YxPu+Sfr6UPqM9LszZvgsWvT/qp4I+DUHHvWY01T40Xa6/zj5qvmQnVj4xUiy+9Q6BPsrBw72x9c4+JjOyv3o1O75045k+Fp2UvW3DDT/1PWW/EGCiv1sCGr7qlsE+IGnFv8Dmub4Qw9Q/JvS1PokSDMAoHiO/WlE0PzdKLj/Qlj0/IOFePxFhhT/8N+s/CWDgPgi3ub9WIZm+QI+aPbZDC79IxWs/ua+VPwaKYD3RA/6+y52HP7H0yz84bMw+Iyvavz8J674naJY/LZeFPYobEL+tEX4/33+uPxePcj7SV0y+QXxFPt1Ckr71dsY8huspv7GZPz/Vv2E/sWmIPywH0D9HGqo+8ZP4v8dWEL+z8Xw/toysP1zFYz7yziu+DUs0PhJoI74P+0Q+wp5TvsIThD5CAKC9yJsgP+3VU793VYm/7pcFwFxFHL9RC0I/0ewvPwJWXD9KmWQ/JKu4P1z2XT6Hh3O+mm4MPRJZCr/TAoc/Co33Pxd1Jj9wUGG/8Ghwv0CJsL8kHoG+YMNoPHdIAb890Gc/6LO/P6rl3b4ma68/J/lTPs1wXL54S2k+hwFEvoYFiz4Ngd690izPPkdhrb/G41W+yX1EPovmob7N3Ig9tkULv2EvZT8OBKc/08C7PQCa3b7KSrs/Li2RPvovYLyykxs/WclVvz7vkr83F/W/9wARvwx5dj80zrk/KHeVPlqbqryZeCI/s/ctv5p2P79dQ16/byOCv8A34L8zk/S+ECWnP+ow6T2jrOy+yjSXP03Ccj2rwA6/TFl8PxMqrD/YO24+w8VUvp7yIj6rZYW+5C8SPh94yb56q9Y/yjDtPsf6lb/kv5a9UWAMP5QhZr8Jk52/xcEHvr2Xqj62NPO/xcsKvz8/cj8ar5k/uluEPLTnFL9eOVE/mwdtP8YZlz9W/5k9w6kIv/mdbz8xVJg/U1wbPPQXHb+v/14/F+iDPxUP5T8k2tg+WjCrv9HYWL4s/TM+niB4vmN5zT2uy/W+5nCiP7k1ID5fZ7q+huDTP6pDsT6oCynAZF4Zv+6xST96dXk/duXDP+O8jD4xhQK+3MOvPul3MsB0fhy/OSRcP2Imgj+fnts/edTtPsn8k79PFzy9ieMEP+YfkL+/5Pi/gdcPv1OVfT+8uq0/1tNsPtJ7Vr6qPDE+Y8x6vmLR0D3qh/W+axuiP7ezGz6F5b2+xsnMP2TqzT5Vr9a/MFTkvkrynD/tchI9Bicev38RPj8SwEI/xaMxP8bzXz8K4Gs/YPOqP+eOlT6Rdre9uUOrPt3zx7+nUuy+hxmrP6C7lj4tDqy9odayPkKu4r9fmKS+aKXTP5Dh3D6xhqW/3/iKvCpS/z561mu/VEerv5WOl74/B6I9wbS2vrfg2j+T68E+ZGPvv4GGIL/qzZw/C0KXPRrT+76rX4s/85H+PxV9ED+IEWa/k5qcvwCWgr26Wws/+dxjvzL3pL9lseu9j8LvPldhoL+TGwC+CCyoPqjr+r+QbAq/sCBoP/BooT+A8AU+2FeuvrCBDUDPnyc/crQ6vy3TYr/JKoO/1Ozmv5h33b6Y9Lk/Gx6WPjeZpLyWNyk/9TpEv7sUUr/kknC/rmyZv2xkeryl0hg/g45Mv+IVe7+v+6y/oP9ovpt+Sz7W+zm+zeecPv22mb3kgAw/+Jtlv2RLn7+MIwC+n7ezPomVCsDK5yS/ArgwP/7eQz+SGFQ/9TNrP3XNlz9b8nk9/WgOv5opfD/S7Kw/BU5wPoPZSb6Pe0Q+8GCWvs5LWzte7B+/+pdWP6zMjT9BZUZA+pQdP8azWL+9GJC/jkL7vwFaE7/+N3Q/UU62P0yekT6ewce8KxQsP0ulO78BQ1q/DjySv1NbBsBcdgS/lumPP9aJ+T98gBA/7Kt9v9jCrr+oMG++89tRPpKNNr7OtqE+ugNIvZUFBD9ylZG/x/UDwEeTAb+TWoI/3JrcP3Ro8T4i2aW/PZ/LvQYN+D6LHaC/5YAKvonDpD4T7QXAfIoEv1CdkD/+M/0/xE4TPzsfdb/iorq/+ZOdvmzecT1y8BO/E1pzP3kUtz9kzpI+e2SwvIl3Kz9IBDy//i9av4Uhg78VFhjAd9MSv0oljD/dKOE/69nNPhhSCsDNGAm/SfWIP7hvBEDecRo/BXFGv01QOb919FO/EeSIv1tLBcCpNhu/YgRFPw7yNT+Aj0w/heOQP7fHzj+3aNE+/nOav55oxL1sUaM+BW/Uv+9p4L7xsqA/Q4BxPcaRK7/Zxlc/pVhEvxXWXb8CZ42/aSP0v4gGBb+62oI/V2TVP/5TwT7MY92/STbnvlSIlT+UxIw9Y9wRv9NKfz+q+6o/o0RpPladb76k5mk+Ti5gvmZsTD4snSe+i6h9PhBl1r3Jhe4+ibibv42NmrwQWh0/77xev7quhr9t/sy/N26zvqfBJkDAXR4//IhVv8t1jb/rxBzAeiEYvwfqRj8RAng/WVvDP2p3hD7DUBa+cVvJPuLs2L+Vduq+eHaUP6HsUD1edfy+tD2HP91Sxj++LL4+MY7Ev1uSwb5bXMk/hyTNPqhO4b9lhfC+KnOlPxCBpz0GZd2+WpDDPworhD4I/w++jr/HPsII6b9c7t2+yCO/PxeDgj4cjrO975fcPuDsvL9zJXi+ZBrQPc9r+b6WOp8/CdkJPrNopb7uDQlAp50GPyVijb+ZA0/Ar2sev6KZWz8bRpE/4AkCQCyCAT+HqIK/gzHcv4FM8b769aY/yL3TPQTl8r4gJaU/LhHGPWUaz76DE7U/bjZHPn0wk74uQcg8rNAqvyq6PT/XO18/XeuEP5jN6T/Snts+o96/v46kgb7I87M9rsfZvoGowj/uj4k+/FAMvocTpD6yEgfAOMEFvwM1jj+Ww05Ag34fP0+5Vr9Vkoy/gdYdwKjaFb+TulA/PMFpPzRApz/WFt09RujuvlZbkz9j9hw9og4Jv9sfiT87hwVAJOIbP2peQ79otzK/v5BivxtPcb9tCrO/56yHvg9Chz0prSi/pW5eP0Y+aD/4yb4/UdkrPpn1Mr4/ICU+IwFDvmxFVj6CtIO+PPuEO0SNBr/wvHY/Cr+jP74mDjw3rwO/TcZ+P1TgmD9OZ7E9gtOtvr/ZED6WILS+hIf3PwXsAT9Zz5K/YsgRwPx5Fb8iTVg/4MqJP2aJzD9ozsI+5GbQv0aUtL4S/EdATLEXP/2HUb+L0WO/CGGov8R2p72NwdE+oYWsv1qFVr6LMz0+Y4iUvsuL6DyqnSS/nQYzP3T+Nj/w1TA/ggc/P/I5Vj9V9JI/k8n3PxaoEj8A2nK/Ph25v8Atnr73CTc9YnAHvxW9jT+S8RpAJwoYP+jIR79QyHe/LDzCv4/oib5qhvk9xEayvlcYREABFBw/E+lfvw8Fib9yic+/4cqlvqiCBUAnPQY/6luKv68XGsDj0ie/1cZBPw3jSD//M3U/8pa4P6jAjz5WW6i8rXoqP2mpP79muGC/mgOGvzqGx79z2r6+HHzJP3+sxz6AZua/5gHWvuCVrj93S2o+I01XvnBzMj64xXW+b9rpPbkr5r50K50/eKbyPDf8I7/hXzA/lllAP5A/SD+QpXQ/oVS6P7DKmz6JyYO9OfEPPy3ufL/bZqy/pJdfvsc6cD4nh1O+m/E5PiVNoL4jc1o9iEsBv+0OhD+Ij+Q/CzDUPt+Lrb8ZzGi+lmhcPvwiJb4JNok+IQcLvlNvpT7DhATAot8Cv1L9kj8g9ghAU30HP/p0i79aLxPAS00ov8TuQj+djEw/ohOAP5+gsT9aljI+GHx4vsK62z3wxO++b+aTP/EP/D1u/N2+oWyjP7uqXDsCqAa/AoB3P4ddpD+6dSY8FXICv/jCZD/3gLg/GTZZPqL9er6koLY8uFsSv34rjT9xqec//xitPpnXxb9ymOi+h0avP6m7dD4ApBa9DpoIP7jxiL8ZOgbAx3Ycvz1HQz8ubTE/d5dgPx1Kbj8="


def _u0():
    return np.frombuffer(base64.b64decode(_U0_B64), dtype="<f4").reshape(5, 256).copy()


def _mm256p(nc, ps512, terms):
    """[256,256] matmul sum into ONE [128,512] psum bank as a single
    accumulation group: result rows m*128..+128 at cols m*256..+256."""
    n = 0
    total = len(terms) * 4
    for m in range(2):
        for lhsT_tiles, rhs_tiles in terms:
            for kt in range(2):
                nc.tensor.matmul(
                    ps512[:, m * 256:(m + 1) * 256],
                    lhsT_tiles[kt][:, m * 128:(m + 1) * 128],
                    rhs_tiles[kt][:],
                    start=n == 0,
                    stop=n == total - 1,
                )
                n += 1


def _mm256(nc, psums, terms):
    """[256,256] matmul sum over terms: psums[m] += sum_p lhsT_p.T @ rhs_p."""
    for m in range(2):
        for pi, (lhsT_tiles, rhs_tiles) in enumerate(terms):
            for kt in range(2):
                nc.tensor.matmul(
                    psums[m][:],
                    lhsT_tiles[kt][:, m * 128:(m + 1) * 128],
                    rhs_tiles[kt][:],
                    start=pi == 0 and kt == 0,
                    stop=pi == len(terms) - 1 and kt == 1,
                )


def _mv256(nc, ps2, lhsT_tiles, z_tiles):
    """matvec into one [128,2] psum tile: half m lands in column m."""
    n = 0
    for m in range(2):
        for kt in range(2):
            nc.tensor.matmul(
                ps2[:, m:m + 1],
                lhsT_tiles[kt][:, m * 128:(m + 1) * 128],
                z_tiles[kt],
                start=n == 0,
                stop=n == 3,
            )
            n += 1


def build_nc(use_cc=USE_CC):
    nc = bacc.Bacc("TRN2", target_bir_lowering=False, debug=False,
                   num_devices=NCORES)

    n_mat_local = 1 if use_cc else NUM_K

    x_in = nc.dram_tensor("x", [BPC, C, H + 2, W + 2], BF16,
                           kind="ExternalInput")
    pm_in = nc.dram_tensor("pm", [n_mat_local, C, C], F32, kind="ExternalInput")
    pmT_in = nc.dram_tensor("pmT", [n_mat_local, C, C], F32, kind="ExternalInput")
    u0_in = nc.dram_tensor("u0", [n_mat_local, C, 1], F32, kind="ExternalInput")
    eye_in = nc.dram_tensor("eye1", [C, C], F32, kind="ExternalInput")
    eyec_in = nc.dram_tensor("eyecat", [128, 512], F32, kind="ExternalInput")
    bias_in = nc.dram_tensor("biasc", [C, 1], F32, kind="ExternalInput")
    out_dram = nc.dram_tensor("out", [BPC, C, H, W], F32, kind="ExternalOutput")

    with tile.TileContext(nc) as tc, contextlib.ExitStack() as top:
        const = top.enter_context(tc.tile_pool(name="const", bufs=1))
        xpool = top.enter_context(tc.tile_pool(name="xpool", bufs=1))
        tpool = top.enter_context(tc.tile_pool(name="tpool", bufs=1))
        v5pool = top.enter_context(tc.tile_pool(name="v5pool", bufs=1))
        vfin = top.enter_context(tc.tile_pool(name="vfin", bufs=1))

        eye = [const.tile([128, 256], F32, name=f"eye_{t}", tag=f"eye{t}")
               for t in range(2)]
        bias_c = [const.tile([128, 1], F32, name=f"bias_{t}", tag=f"bias{t}")
                  for t in range(2)]
        ones_f = const.tile([1, 128], F32, name="ones_f", tag="onesf")
        eyec = const.tile([128, 512], F32, name="eyec", tag="eyec")
        nc.sync.dma_start(eyec[:], eyec_in[:])
        for t in range(2):
            nc.sync.dma_start(eye[t][:], eye_in[t * 128:(t + 1) * 128, :])
            nc.sync.dma_start(bias_c[t][:], bias_in[t * 128:(t + 1) * 128, :])
        nc.any.memset(ones_f[:], 1.0)

        # ---- chain inputs staged first so their DMAs beat the big x DMAs ----
        chain_in = []
        cinp = top.enter_context(tc.tile_pool(name="cinp", bufs=1))
        for mi in range(n_mat_local):
            A = [cinp.tile([128, 256], F32, name=f"A_{mi}_{t}", tag=f"A{mi}{t}")
                 for t in range(2)]
            AT = [cinp.tile([128, 256], F32, name=f"AT_{mi}_{t}", tag=f"AT{mi}{t}")
                  for t in range(2)]
            z0 = [cinp.tile([128, 1], F32, name=f"z0_{mi}_{t}", tag=f"z0{mi}{t}")
                  for t in range(2)]
            for t in range(2):
                nc.sync.dma_start(A[t][:], pm_in[mi, t * 128:(t + 1) * 128, :])
                nc.sync.dma_start(AT[t][:], pmT_in[mi, t * 128:(t + 1) * 128, :])
                nc.sync.dma_start(z0[t][:], u0_in[mi, t * 128:(t + 1) * 128, :])
            chain_in.append((A, AT, z0))

        # per-stage scaled-identity lhsT for the b*G psum term:
        #   bI[i] = (b_i/sqrt(c_i)) * I128
        aeye0 = const.tile([128, 512], F32, name="aeye0", tag="aeye0")
        nc.scalar.mul(aeye0[:], eyec[:], float(SCHED[0][0]))
        bI = []
        for i, (a_i, b_i, c_i) in enumerate(SCHED):
            bi = const.tile([128, 128], F32R, name=f"bI_{i}", tag=f"bI{i}")
            nc.vector.tensor_scalar_mul(bi[:], eye[0][:, 0:128],
                                        float(b_i / math.sqrt(c_i)))
            bI.append(bi)

        # ---- early barrier: tiny AllGather aligns core start times so the
        # real gather's multi-phase mesh sees ~zero skew ---------------------
        barp = top.enter_context(tc.tile_pool(name="barp", bufs=1,
                                              space="DRAM"))
        if use_cc:
            # fire-and-forget: forces CC-ring/peer init to overlap the chain
            # so the real gather is the (fast) second collective
            bsrc = const.tile([128, 1], F32, name="bsrc", tag="bsrc")
            nc.vector.memset(bsrc[:], 1.0)
            bgin = barp.tile([128, 1], F32, name="bgin", tag="bgin")
            bgout = barp.tile([NCORES, 128, 1], F32, name="bgout", tag="bgout")
            nc.sync.dma_start(bgin[:], bsrc[:])
            nc.gpsimd.collective_compute(
                "AllGather", mybir.AluOpType.bypass,
                replica_groups=[list(range(NCORES))],
                ins=[bgin.opt()], outs=[bgout.opt()],
            )

        # ---- PE warmup burst (gated on the barrier) to lift the clock ------
        wsrc = const.tile([128, 512], F32, name="wsrc", tag="wsrc")
        nc.vector.memset(wsrc[:], 1.0)
        dummy2 = const.tile([128, 512], F32R, name="dummy2", tag="dummy2")
        nc.vector.tensor_copy(dummy2[:], wsrc[:])
        dummy_r = const.tile([128, 256], F32R, name="dummy_r", tag="dummyr")
        nc.scalar.copy(dummy_r[:], wsrc[:, 0:256])
        with tc.tile_pool(name="warmps", bufs=1, space="PSUM") as wps:
            wp_ = wps.tile([128, 512], F32, name="warm", tag="warm")
            for wi in range(NWARM):
                nc.tensor.matmul(wp_[:], dummy_r[:, 0:128], dummy2[:],
                                 start=wi == 0, stop=wi == NWARM - 1)

        # ---- x load: direct bf16 DMA into padded tile + tiny edge copies ----
        Xp = [[xpool.tile([128, 66, 66], BF16, name=f"xp_{b}_{g}",
                          tag=f"xp{b}{g}") for g in range(2)]
              for b in range(BPC)]
        for b in range(BPC):
            for g in range(2):
                nc.sync.dma_start(Xp[b][g][:],
                                  x_in[b, g * 128:(g + 1) * 128, :, :])

        # ---- per-matrix chain: power iteration + fitted quintic schedule ----
        def half(tile512):
            return [tile512[:, t * 256:(t + 1) * 256] for t in range(2)]

        def fill(tag_idx, n, free=256):
            for fi in range(n):
                fl = cps.tile([1, free], F32, name="fl", tag="fl0", bufs=1)
                nc.tensor.matmul(fl[:], dummy_r[0:128, 0:1],
                                 dummy2[:, 0:free], start=True, stop=True)

        def mmh(ps_half, m, lhsT_tiles, rhs_tiles, extra_bI=None):
            """Half-row product: ps_half = (lhsT^T rhs) rows m*128..+128,
            optionally + bI^T rhs_m as a 3rd matmul in the same group."""
            total = 2 + (1 if extra_bI is not None else 0)
            for kt in range(2):
                nc.tensor.matmul(ps_half[:],
                                 lhsT_tiles[kt][:, m * 128:(m + 1) * 128],
                                 rhs_tiles[kt][:],
                                 start=kt == 0, stop=kt == 1 and total == 2)
            if extra_bI is not None:
                nc.tensor.matmul(ps_half[:], extra_bI[:], rhs_tiles[m][:],
                                 start=False, stop=True)

        v_final = []
        with tc.tile_pool(name="chain", bufs=2) as cp, \
             tc.tile_pool(name="chps", bufs=2, space="PSUM") as cps:
            for mi in range(n_mat_local):
                pt = mi % 2
                A, AT, z0 = chain_in[mi]

                # fp32r copies of A / A^T (used by PI and as stage-0 W/V)
                Ar_sb = cp.tile([128, 512], F32R, name="Ar", tag=f"Ar{pt}",
                                bufs=1)
                ATr_sb = cp.tile([128, 512], F32R, name="ATr", tag=f"ATr{pt}",
                                 bufs=1)
                for t in range(2):
                    nc.vector.tensor_copy(Ar_sb[:, t * 256:(t + 1) * 256],
                                          A[t][:])
                    nc.scalar.copy(ATr_sb[:, t * 256:(t + 1) * 256], AT[t][:])
                Ar, ATr = half(Ar_sb), half(ATr_sb)

                # Deferred-normalization power iteration (replicates the
                # reference's 10 normalized PI rounds up to fp noise):
                # G2 = A A^T, G4 = G2^2, G8 = G4^2; z9 = G8 G2 z0;
                # y = A^T z9; zf = A y; s = sqrt(|zf|^2/|y|^2).
                # Stage-0's unscaled G-hat = A^T A is interleaved here; the
                # 1/s^2 and 1/s factors fold into stage-0's copies.
                g2h = [cps.tile([128, 256], F32, name=f"g2_{m}",
                                tag=("hA", "hB")[m]) for m in range(2)]
                for m in range(2):
                    mmh(g2h[m], m, AT, AT)
                G2_sb = cp.tile([128, 512], F32, name="G2_sb", tag=f"G2{pt}",
                                bufs=1)
                nc.scalar.copy(G2_sb[:, 0:256], g2h[0][:])
                nc.vector.tensor_copy(G2_sb[:, 256:512], g2h[1][:])
                G2 = half(G2_sb)
                # stage-0 G-hat (independent of the scale chain)
                gph = [cps.tile([128, 256], F32, name=f"gp_{m}",
                                tag=("hC", "hD")[m], bufs=1) for m in range(2)]
                for m in range(2):
                    mmh(gph[m], m, Ar, Ar)
                g4h = [cps.tile([128, 256], F32, name=f"g4_{m}",
                                tag=("hA", "hB")[m]) for m in range(2)]
                for m in range(2):
                    mmh(g4h[m], m, G2, G2)
                Gu_sb = cp.tile([128, 512], F32R, name="Gu_sb",
                                tag=f"Gu{pt}", bufs=1)
                nc.scalar.copy(Gu_sb[:, 0:256], gph[0][:])
                nc.vector.tensor_copy(Gu_sb[:, 256:512], gph[1][:])
                Gu = half(Gu_sb)
                G4_sb = cp.tile([128, 512], F32, name="G4_sb", tag=f"G4{pt}",
                                bufs=1)
                nc.scalar.copy(G4_sb[:, 0:256], g4h[0][:])
                nc.vector.tensor_copy(G4_sb[:, 256:512], g4h[1][:])
                G4 = half(G4_sb)
                # z1 = G2 z0 while G8 is on the PE
                zp = cps.tile([128, 2], F32, name="zp", tag="pv0", bufs=1)
                _mv256(nc, zp, G2, z0)
                zs = cp.tile([128, 2], F32, name="zs", tag=f"z{pt}")
                nc.vector.tensor_copy(zs[:], zp[:])
                z = [zs[:, 0:1], zs[:, 1:2]]
                g8h = [cps.tile([128, 256], F32, name=f"g8_{m}",
                                tag=("hA", "hB")[m]) for m in range(2)]
                for m in range(2):
                    mmh(g8h[m], m, G4, G4)
                G8_sb = cp.tile([128, 512], F32, name="G8_sb", tag=f"G8{pt}",
                                bufs=1)
                nc.scalar.copy(G8_sb[:, 0:256], g8h[0][:])
                nc.vector.tensor_copy(G8_sb[:, 256:512], g8h[1][:])
                G8 = half(G8_sb)
                g2u = [cps.tile([128, 256], F32, name=f"g2u_{m}",
                                tag=("hA", "hB")[m]) for m in range(2)]
                for m in range(2):
                    mmh(g2u[m], m, Gu, Gu)
                fill(0, PI_FILL)
                # z9 = G8 z1 ; y = A^T z9 ; zf = A y
                zp = cps.tile([128, 2], F32, name="zp9", tag="pv0", bufs=1)
                _mv256(nc, zp, G8, z)
                zs = cp.tile([128, 2], F32, name="zs9", tag=f"z9{pt}")
                nc.vector.tensor_copy(zs[:], zp[:])
                z = [zs[:, 0:1], zs[:, 1:2]]
                fill(1, 2)
                yp = cps.tile([128, 2], F32, name="yp", tag="pv0", bufs=1)
                _mv256(nc, yp, A, z)
                ys = cp.tile([128, 2], F32, name="ys", tag=f"y{pt}")
                nc.vector.tensor_copy(ys[:], yp[:])
                y = [ys[:, 0:1], ys[:, 1:2]]
                fill(0, 2)
                zp = cps.tile([128, 2], F32, name="zfp", tag="pv0", bufs=1)
                _mv256(nc, zp, AT, y)
                zs = cp.tile([128, 2], F32, name="zfs", tag=f"zf{pt}")
                nc.vector.tensor_copy(zs[:], zp[:])
                z = [zs[:, 0:1], zs[:, 1:2]]
                fill(1, 2)

                # ny = y.y ; nz = z.z ; inv_s = sqrt(ny/nz) bcast;
                # inv_s2 = ny/nz bcast
                nyp = cps.tile([128, 1], F32, name="nyp", tag="pv0", bufs=1)
                for t in range(2):
                    nc.tensor.matmul(nyp[0:1, 0:1], y[t], y[t],
                                     start=t == 0, stop=t == 1)
                nzp = cps.tile([128, 1], F32, name="nzp", tag="pv0", bufs=1)
                for t in range(2):
                    nc.tensor.matmul(nzp[0:1, 0:1], z[t], z[t],
                                     start=t == 0, stop=t == 1)
                sc = cp.tile([1, 6], F32, name="sc", tag=f"sc{pt}")
                nc.vector.tensor_copy(sc[:, 0:1], nyp[0:1, :])
                nc.vector.reciprocal(sc[:, 1:2], nzp[0:1, :])
                nc.vector.tensor_scalar_mul(sc[:, 2:3], sc[:, 0:1],
                                            sc[:, 1:2])
                nc.scalar.sqrt(sc[:, 3:4], sc[:, 2:3])
                nc.vector.tensor_scalar_mul(sc[:, 4:5], sc[:, 2:3],
                                            float(SCHED[0][1]))
                nc.vector.tensor_scalar(sc[:, 5:6], sc[:, 2:3], sc[:, 2:3],
                                        float(SCHED[0][2]),
                                        op0=mybir.AluOpType.mult,
                                        op1=mybir.AluOpType.mult)
                bcp = cps.tile([128, 3], F32, name="bcp", tag="pv0", bufs=1)
                nc.tensor.matmul(bcp[:], ones_f[:], sc[:, 3:6],
                                 start=True, stop=True)
                inv_b = cp.tile([128, 3], F32, name="inv_b", tag=f"invb{pt}")
                nc.scalar.copy(inv_b[:], bcp[:])
                inv_s = inv_b[:, 0:1]    # 1/s broadcast
                b0_bc = inv_b[:, 1:2]    # b_0/s^2 broadcast
                c0_bc = inv_b[:, 2:3]    # c_0/s^4 broadcast

                w, v = Ar, ATr
                for it in range(NSTAGE):
                    a_i, b_i, c_i = SCHED[it]
                    sq_i = float(math.sqrt(c_i))
                    last = it == NSTAGE - 1
                    if it == 0:
                        gph_it = gph   # precomputed A^T A (unscaled)
                    else:
                        gph_it = [cps.tile([128, 256], F32, name=f"gp_{m}",
                                           tag=("hC", "hD")[m], bufs=1)
                                  for m in range(2)]
                        for m in range(2):
                            mmh(gph_it[m], m, w, w)
                    M_sb = cp.tile([128, 512], F32R, name="M_sb", tag=f"M{pt}")
                    if it == 0:
                        # M = a0*I + (b0/s^2)*Ghat + (c0/s^4)*Ghat^2, built
                        # straight from the unscaled psums with runtime bcasts
                        u1 = cp.tile([128, 512], F32, name="u1",
                                     tag=f"u1{pt}", bufs=1)
                        for m in range(2):
                            h = slice(m * 256, (m + 1) * 256)
                            nc.vector.scalar_tensor_tensor(
                                u1[:, h], gph_it[m][:], b0_bc, aeye0[:, h],
                                op0=mybir.AluOpType.mult,
                                op1=mybir.AluOpType.add)
                            nc.vector.scalar_tensor_tensor(
                                M_sb[:, h], g2u[m][:], c0_bc, u1[:, h],
                                op0=mybir.AluOpType.mult,
                                op1=mybir.AluOpType.add)
                    else:
                        Gs_sb = cp.tile([128, 512], F32R, name="Gs_sb",
                                        tag=f"G{pt}")
                        nc.scalar.mul(Gs_sb[:, 0:256], gph_it[0][:], sq_i)
                        nc.vector.tensor_scalar_mul(Gs_sb[:, 256:512],
                                                    gph_it[1][:], sq_i)
                        Gs = half(Gs_sb)
                        fill(it, STAGE_FILL_A)
                        # mp = c*G^2 + b*G per half (separate groups)
                        mph = [cps.tile([128, 256], F32, name=f"mp_{m}",
                                        tag=("hA", "hB")[m]) for m in range(2)]
                        for m in range(2):
                            mmh(mph[m], m, Gs, Gs, extra_bI=bI[it])
                        # M = a_i*eyec + mp per half
                        nc.vector.scalar_tensor_tensor(
                            M_sb[:, 0:256], eyec[:, 0:256], float(a_i),
                            mph[0][:],
                            op0=mybir.AluOpType.mult, op1=mybir.AluOpType.add)
                        nc.vector.scalar_tensor_tensor(
                            M_sb[:, 256:512], eyec[:, 256:512], float(a_i),
                            mph[1][:],
                            op0=mybir.AluOpType.mult, op1=mybir.AluOpType.add)
                    M = half(M_sb)
                    fill(it + 1, STAGE_FILL_B)
                    if last and use_cc:
                        # Only V'[0:128,:] feeds the gather.
                        vph = cps.tile([128, 256], F32, name="vp2", tag="hC",
                                       bufs=1)
                        for kt in range(2):
                            nc.tensor.matmul(vph[:],
                                             M[kt][:, 0:128], v[kt],
                                             start=kt == 0, stop=kt == 1)
                        v_sb = vfin.tile([128, 512], F32R, name=f"vf_{mi}",
                                         tag=f"vf{mi}")
                        nc.vector.tensor_copy(v_sb[:, 0:256], vph[:])
                        v = half(v_sb)
                        v_final.append(v)
                        break
                    # W' = W M (lhsT = V), V' = M V (lhsT = M), per half
                    wph = [cps.tile([128, 256], F32, name=f"wp_{m}",
                                    tag=("hA", "hB")[m]) for m in range(2)]
                    vph = [cps.tile([128, 256], F32, name=f"vp_{m}",
                                    tag=("hC", "hD")[m], bufs=1) for m in range(2)]
                    for m in range(2):
                        mmh(wph[m], m, v, M)
                    for m in range(2):
                        mmh(vph[m], m, M, v)
                    fill(it, 2)
                    pool = vfin if last else cp
                    w_sb = cp.tile([128, 512], F32R, name="w_sb", tag=f"w{pt}")
                    v_sb = pool.tile([128, 512], F32R,
                                     name=f"vf_{mi}" if last else "v_sb",
                                     tag=f"vf{mi}" if last else f"v{pt}")
                    if it == 0:
                        nc.scalar.mul(w_sb[:, 0:256], wph[0][:], inv_s)
                        nc.vector.tensor_scalar_mul(w_sb[:, 256:512],
                                                    wph[1][:], inv_s)
                        nc.scalar.mul(v_sb[:, 0:256], vph[0][:], inv_s)
                        nc.vector.tensor_scalar_mul(v_sb[:, 256:512],
                                                    vph[1][:], inv_s)
                    else:
                        nc.scalar.copy(w_sb[:, 0:256], wph[0][:])
                        nc.vector.tensor_copy(w_sb[:, 256:512], wph[1][:])
                        nc.scalar.copy(v_sb[:, 0:256], vph[0][:])
                        nc.vector.tensor_copy(v_sb[:, 256:512], vph[1][:])
                    w, v = half(w_sb), half(v_sb)
                else:
                    v_final.append(v)

        # ---- gather the needed V halves across cores ------------------------
        # Rank r carries matrix [1,2,3,4, 0, 0*Q, 1, 2][r], where Q swaps
        # column halves - ortho(A Q) = ortho(A) Q, so rank 5's V[:128]
        # equals V_0[128:256].
        V0 = [None, None]
        Vh = [None] * 4
        if use_cc:
            with tc.tile_pool(name="ccdram", bufs=1, space="DRAM") as dp, \
                 tc.tile_pool(name="vstg", bufs=4) as vstg, \
                 tc.tile_pool(name="fillps", bufs=2, space="PSUM") as fps:
                gin = dp.tile([128, C], BF16, name="gin", tag="gin")
                gout = dp.tile([NCORES, 128, C], BF16, name="gout", tag="gout")
                vbf = vstg.tile([128, C], BF16, name="vbf", tag="vbf")
                nc.vector.tensor_copy(vbf[:], v_final[0][0])
                nc.sync.dma_start(gin[:, :], vbf[:])
                gate = vstg.tile([128, 1], F32R, name="gate", tag="gate")
                nc.vector.tensor_copy(gate[:], v_final[0][0][:, 0:1])
                nc.gpsimd.collective_compute(
                    "AllGather", mybir.AluOpType.bypass,
                    replica_groups=[list(range(NCORES))],
                    ins=[gin.opt()], outs=[gout.opt()],
                )
                for gi in range(GFILL_GROUPS):
                    fl = fps.tile([1, 512], F32, name=f"gfill_{gi}",
                                  tag=f"gfill{gi % 2}")
                    for wi in range(6):
                        nc.tensor.matmul(fl[:], gate[:, 0:1], dummy2[:],
                                         start=wi == 0, stop=wi == 5)
                for slot, dest in [(0, ("vh", 0)), (1, ("vh", 1)),
                                   (2, ("vh", 2)), (3, ("vh", 3)),
                                   (4, ("v0", 0)), (5, ("v0", 1))]:
                    kind, idx = dest
                    vs = v5pool.tile([128, 256], BF16, name=f"{kind}_{idx}",
                                     tag=f"{kind}{idx}")
                    nc.sync.dma_start(vs[:], gout[slot, :, :])
                    if kind == "vh":
                        Vh[idx] = vs
                    else:
                        V0[idx] = vs
        else:
            for t in range(2):
                v0 = v5pool.tile([128, 256], BF16, name=f"v50_{t}", tag=f"v50{t}")
                nc.vector.tensor_copy(v0[:], v_final[0][t])
                V0[t] = v0
            for b in range(4):
                vh = v5pool.tile([128, 256], BF16, name=f"vh_{b}", tag=f"vh{b}")
                nc.vector.tensor_copy(vh[:], v_final[1 + b][0])
                Vh[b] = vh

        # ---- tail: PQ, block_orth pair products, matrix_conv, T -------------
        Ttap = [[[tpool.tile([128, 256], BF16, name=f"T_{k}_{l}_{t}",
                             tag=f"T{k}{l}{t}")
                  for t in range(2)] for l in range(3)] for k in range(3)]
        with tc.tile_pool(name="tail", bufs=1) as tl, \
             tc.tile_pool(name="tailps", bufs=1, space="PSUM") as tps:
            if use_cc:
                for wi in range(2):
                    twf = tps.tile([1, 256], F32, name=f"twf_{wi}",
                                   tag="twf0")
                    nc.tensor.matmul(twf[:], Vh[0][:, 0:1], Vh[0][:],
                                     start=True, stop=True)
            PQ = []
            for b in range(4):
                pq = [tl.tile([128, 256], F32R, name=f"pq_{b}_{t}",
                              tag=f"pq{b}{t}") for t in range(2)]
                ps = tps.tile([128, 512], F32, name="pqps", tag="pqps")
                for m in range(2):
                    nc.tensor.matmul(ps[:, m * 256:(m + 1) * 256],
                                     Vh[b][:, m * 128:(m + 1) * 128],
                                     Vh[b][:],
                                     start=True, stop=True)
                for m in range(2):
                    nc.scalar.copy(pq[m][:], ps[:, m * 256:(m + 1) * 256])
                del ps
                PQ.append(pq)

            def pair_products(pa, pb, name):
                """e[c][r]: [0][0]=pa@pb, [0][1]=pa-C, [1][0]=pb-C,
                [1][1]=I-pa-pb+C (symmetric projection algebra)."""
                ps = [tps.tile([128, 256], F32, name=f"ccps_{t}", tag=f"ccps{t}")
                      for t in range(2)]
                _mm256(nc, ps, [(pa, pb)])
                e = [[[tl.tile([128, 256], F32R, name=f"{name}_e{i}{j}_{t}",
                               tag=f"{name}e{i}{j}{t}")
                       for t in range(2)] for j in range(2)] for i in range(2)]
                q = [tl.tile([128, 256], F32, name=f"{name}_q_{t}",
                             tag=f"{name}q{t}") for t in range(2)]
                for t in range(2):
                    nc.scalar.copy(e[0][0][t][:], ps[t][:])
                    nc.vector.tensor_sub(e[0][1][t][:], pa[t][:],
                                         e[0][0][t][:].bitcast(F32))
                    nc.vector.tensor_sub(e[1][0][t][:], pb[t][:],
                                         e[0][0][t][:].bitcast(F32))
                    nc.vector.tensor_sub(q[t][:], eye[t][:],
                                         pa[t][:].bitcast(F32))
                    nc.vector.tensor_sub(e[1][1][t][:], q[t][:],
                                         e[1][0][t][:].bitcast(F32))
                return e

            # m1T[c1][r1] = a2[c1] a1[r1]; m2[r2][c2] = a3[r2] a4[c2]
            m1T = pair_products(PQ[1], PQ[0], "m1T")
            m2 = pair_products(PQ[2], PQ[3], "m2")

            with tc.tile_pool(name="p3pool", bufs=3) as p3p:
                for i in range(3):
                    for j in range(3):
                        terms = [(i1, j1) for i1 in range(min(2, i + 1))
                                 for j1 in range(min(2, j + 1))
                                 if i - i1 < 2 and j - j1 < 2]
                        ps = [tps.tile([128, 256], F32, name=f"p3ps_{t}",
                                       tag=f"p3ps{t}") for t in range(2)]
                        _mm256(nc, ps, [(m1T[j1][i1], m2[i - i1][j - j1])
                                        for (i1, j1) in terms])
                        cell = [p3p.tile([128, 256], BF16, name=f"cell_{t}",
                                         tag=f"cell{t}") for t in range(2)]
                        for t in range(2):
                            nc.scalar.copy(cell[t][:], ps[t][:])
                        tp = [tps.tile([128, 256], F32, name=f"tps_t{t}",
                                       tag=f"tpsT{t}") for t in range(2)]
                        _mm256(nc, tp, [(V0, cell)])
                        for t in range(2):
                            nc.scalar.copy(Ttap[i][j][t][:], tp[t][:])

        # ---- conv: out[o, pix] += T[kw][kh][i, o] * Xp[i, pix+tap] ----------
        with tc.tile_pool(name="ops", bufs=8, space="PSUM") as ops, \
             tc.tile_pool(name="ostg", bufs=8) as ostg:
            for b in range(BPC):
                for ot in range(2):
                    for q in range(4):
                        ptiles = [ops.tile([128, 512], F32, name=f"cps_{k}",
                                           tag="convps") for k in range(2)]
                        first, last = (0, 0), (8, 1)
                        for tap in range(9):
                            kh, kw = tap // 3, tap % 3
                            for kt in range(2):
                                lhs = Ttap[kw][kh][kt][:, ot * 128:(ot + 1) * 128]
                                for k in range(2):
                                    h0 = q * 16 + k * 8
                                    rhs = Xp[b][kt][:, h0 + kh:h0 + kh + 8,
                                                    kw:kw + 64]
                                    nc.tensor.matmul(
                                        ptiles[k][:], lhs, rhs,
                                        start=(tap, kt) == first,
                                        stop=(tap, kt) == last)
                        for k in range(2):
                            h0 = q * 16 + k * 8
                            so = ostg.tile([128, 512], F32, name="so",
                                           tag="ostg")
                            if k == 0:
                                nc.scalar.activation(
                                    so[:], ptiles[k][:], AF.Identity,
                                    bias=bias_c[ot][:], scale=1.0)
                            else:
                                nc.vector.tensor_scalar_add(
                                    so[:], ptiles[k][:], bias_c[ot][:])
                            nc.sync.dma_start(
                                out_dram[b, ot * 128:(ot + 1) * 128,
                                         h0:h0 + 8, :].rearrange(
                                             "c h w -> c (h w)"),
                                so[:])

    nc.compile()
    return nc


_CACHE = {}


def _get_nc():
    key = (USE_CC,)
    if key not in _CACHE:
        _CACHE[key] = build_nc(USE_CC)
    return _CACHE[key]


def make_in_maps(x, param_matrices, bias, use_cc=None):
    if use_cc is None:
        use_cc = USE_CC
    import ml_dtypes
    x = np.ascontiguousarray(x, dtype=np.float32).astype(ml_dtypes.bfloat16)
    x = np.pad(x, ((0, 0), (0, 0), (1, 1), (1, 1)), mode="wrap")
    pm = np.ascontiguousarray(param_matrices, dtype=np.float32)
    bias = np.ascontiguousarray(bias, dtype=np.float32)
    u0 = _u0()
    eye1 = np.eye(C, dtype=np.float32)
    eyecat = np.zeros((128, 512), np.float32)
    for t in range(2):
        eyecat[:, t * 256 + t * 128:t * 256 + t * 128 + 128] = np.eye(128)
    bias_c = bias.reshape(C, 1)
    pmT = np.ascontiguousarray(pm.transpose(0, 2, 1))
    in_maps = []
    perm = np.concatenate([np.arange(128, 256), np.arange(128)])
    pm0q = np.ascontiguousarray(pm[0][:, perm])
    pm0qT = np.ascontiguousarray(pm0q.T)
    sel_map = [1, 2, 3, 4, 0, -1, 1, 2]   # -1 = matrix 0 * Q
    for c in range(NCORES):
        if use_cc:
            sel = sel_map[c]
            if sel == -1:
                pm_l = pm0q[None]
                pmT_l = pm0qT[None]
                u0_l = u0[0:1, :, None]
            else:
                pm_l = pm[sel:sel + 1]
                pmT_l = pmT[sel:sel + 1]
                u0_l = u0[sel:sel + 1, :, None]
        else:
            pm_l, pmT_l, u0_l = pm, pmT, u0[:, :, None]
        in_maps.append({
            "x": x[c * BPC:(c + 1) * BPC],
            "pm": np.ascontiguousarray(pm_l),
            "pmT": np.ascontiguousarray(pmT_l),
            "u0": np.ascontiguousarray(u0_l),
            "eye1": eye1,
            "eyecat": eyecat,
            "biasc": bias_c,
        })
    return in_maps


def kernel(x, param_matrices, bias, _trace=False):
    nc = _get_nc()
    in_maps = make_in_maps(x, param_matrices, bias)
    res = run_bass_kernel_spmd(nc, in_maps, list(range(NCORES)), trace=_trace)
    out = np.concatenate([res.results[c]["out"] for c in range(NCORES)], axis=0)
    if _trace:
        kernel._last_result = res
    return out
